# revision 1
# baseline (speedup 1.0000x reference)
"""Trainium2 Bass kernel for nn_MultiHeadAttention_85761906966848 (sparse_attention).

The reference module only uses the DIAGONAL of the softmax attention matrix:
    out[b,s,:] = (softmax(masked scores)[s,s] * v[b,s,:]) @ W0 + b0
so no attn @ V matmul is needed — only QK^T row-sums of exp (softmax
denominators), the diagonal q_s.k_s, and the four dense projections.

Facts used:
  * For s < L (=lengths[b]) the pad mask never intersects the causal region,
    so denominators are pure-causal sums over t <= s.
  * For s >= L the diagonal softmax weight is 0, so out rows are exactly b0 —
    implemented by zeroing the diagonal weight with a host-built mask.

Sharding: data-parallel over batch — core b computes batch b end-to-end.
All matmuls run as float32r (full-speed fp32 mode of the PE array); X^T is
pre-transposed on the host; biases/indicator matrices/masks are host-built
constants shaped for the on-chip layouts.
"""

import numpy as np
import concourse.bass as bass
import concourse.bacc as bacc
import concourse.mybir as mybir
from concourse import tile

F32 = mybir.dt.float32
F32R = mybir.dt.float32r
AF = mybir.ActivationFunctionType

B, S, D, H = 8, 1024, 1024, 16
NEG = -1.0e30

_CACHE = {}


def blocks(total, width=512):
    out = []
    off = 0
    while off < total:
        w = min(width, total - off)
        out.append((off, w))
        off += w
    return out


def _build(S=1024, D=1024, H=16):
    dk = D // H
    C = D // 128          # number of 128-row d-chunks
    T = S // 128          # number of 128-row s-tiles
    HPC = 128 // dk       # heads per chunk
    assert dk * H == D and C * 128 == D and T * 128 == S and HPC * dk == 128

    MMDT = F32R

    nc = bacc.Bacc("TRN2", target_bir_lowering=False, debug=False, num_devices=8)

    xt_d = nc.dram_tensor("xt", [D, S], MMDT, kind="ExternalInput")
    w_d = {}
    for wn in ("wq", "wk", "wv", "w0"):
        w_d[wn] = nc.dram_tensor(wn, [D, D], MMDT, kind="ExternalInput")
    bqt_d = nc.dram_tensor("bqt", [128, C], F32, kind="ExternalInput")
    bkt_d = nc.dram_tensor("bkt", [128, C], F32, kind="ExternalInput")
    bvt_d = nc.dram_tensor("bvt", [128, C], F32, kind="ExternalInput")
    b0b_d = nc.dram_tensor("b0b", [128, D], F32, kind="ExternalInput")
    ed_d = nc.dram_tensor("ed", [C, 128, H], MMDT, kind="ExternalInput")
    ebc_d = nc.dram_tensor("ebc", [C, H, 128], MMDT, kind="ExternalInput")
    mask_d = nc.dram_tensor("maskh", [H, S], F32, kind="ExternalInput")
    tril_d = nc.dram_tensor("trilneg", [128, 128], F32, kind="ExternalInput")
    iden_d = nc.dram_tensor("iden", [128, 128], F32, kind="ExternalInput")
    out_d = nc.dram_tensor("out", [S, D], F32, kind="ExternalOutput")

    with tile.TileContext(nc) as tc:
        with (
            tc.tile_pool(name="cp", bufs=1) as cp,
            tc.tile_pool(name="xtp", bufs=1) as xtp,
            tc.tile_pool(name="wp", bufs=C) as wp,
            tc.tile_pool(name="qkp", bufs=1) as qkp,
            tc.tile_pool(name="vp", bufs=1) as vp,
            tc.tile_pool(name="prodp", bufs=2) as prodp,
            tc.tile_pool(name="qzp", bufs=2) as qzp,
            tc.tile_pool(name="outp", bufs=2) as outp,
            tc.tile_pool(name="pp", bufs=2, space=bass.MemorySpace.PSUM) as pp,
            tc.tile_pool(name="pbig", bufs=3, space=bass.MemorySpace.PSUM) as pbig,
        ):
            # ---------------- constants ----------------
            iden = cp.tile([128, 128], F32, tag="iden")
            nc.sync.dma_start(iden[:], iden_d[:])
            tril = cp.tile([128, 128], F32, tag="tril")
            nc.sync.dma_start(tril[:], tril_d[:])
            b0b = cp.tile([128, D], F32, tag="b0b")
            nc.sync.dma_start(b0b[:], b0b_d[:])
            bqt = cp.tile([128, C], F32, tag="bqt")
            nc.sync.dma_start(bqt[:], bqt_d[:])
            bkt = cp.tile([128, C], F32, tag="bkt")
            nc.sync.dma_start(bkt[:], bkt_d[:])
            bvt = cp.tile([128, C], F32, tag="bvt")
            nc.sync.dma_start(bvt[:], bvt_d[:])
            maskh = cp.tile([H, S], F32, tag="maskh")
            nc.sync.dma_start(maskh[:], mask_d[:])
            ed = []
            ebc = []
            for c in range(C):
                e1 = cp.tile([128, H], MMDT, name=f"ed{c}", tag=f"ed{c}")
                nc.sync.dma_start(e1[:], ed_d[c, :, :])
                ed.append(e1)
                e2 = cp.tile([H, 128], MMDT, name=f"ebc{c}", tag=f"ebc{c}")
                nc.sync.dma_start(e2[:], ebc_d[c, :, :])
                ebc.append(e2)

            # persistent small result tiles
            diag_exp = cp.tile([H, S], F32, tag="diag_exp")
            denomT = cp.tile([H, S], F32, tag="denomT")
            arec = cp.tile([H, S], F32, tag="arec")
            a_t = cp.tile([H, S], MMDT, tag="a_t")
            dn = [cp.tile([128, H], F32, name=f"dn{i}", tag=f"dn{i}") for i in range(T)]

            # ---------------- X^T load (host pre-transposed) ----------------
            xt = [xtp.tile([128, S], MMDT, name=f"xt{c}", tag=f"xt{c}") for c in range(C)]
            for (off, wd) in blocks(S):
                for c in range(C):
                    nc.sync.dma_start(xt[c][:, off:off + wd],
                                      xt_d[c * 128:(c + 1) * 128, off:off + wd])

            # ---------------- projections ----------------
            def proj(w_dram, bias_tile, dst_tag, pool):
                wts = [wp.tile([128, D], MMDT, name=f"w{c}", tag="w")
                       for c in range(C)]
                for (off, wd) in blocks(D):
                    for c in range(C):
                        nc.scalar.dma_start(
                            wts[c][:, off:off + wd],
                            w_dram[c * 128:(c + 1) * 128, off:off + wd])
                dst = [pool.tile([128, S], MMDT, name=f"{dst_tag}{c}", tag=f"{dst_tag}{c}")
                       for c in range(C)]
                for dd in range(C):
                    for (off, wd) in blocks(S):
                        ps = pp.tile([128, 512], F32, tag="mm")
                        for kk in range(C):
                            nc.tensor.matmul(
                                ps[:, 0:wd],
                                wts[kk][:, dd * 128:(dd + 1) * 128],
                                xt[kk][:, off:off + wd],
                                start=(kk == 0),
                                stop=(kk == C - 1),
                            )
                        nc.vector.tensor_scalar_add(
                            dst[dd][:, off:off + wd], ps[:, 0:wd],
                            bias_tile[:, dd:dd + 1],
                        )
                return dst

            qt = proj(w_d["wq"], bqt, "q", qkp)
            kt = proj(w_d["wk"], bkt, "k", qkp)

            # ---------------- diag: q_s . k_s per head ----------------
            dg = pbig.tile([H, S], F32, tag="big")
            for (off, wd) in blocks(S):
                for c in range(C):
                    pr = prodp.tile([128, 512], MMDT, tag="prod")
                    nc.vector.tensor_mul(
                        pr[:, 0:wd], qt[c][:, off:off + wd], kt[c][:, off:off + wd])
                    nc.tensor.matmul(
                        dg[:, off:off + wd],
                        ed[c][:],
                        pr[:, 0:wd],
                        start=(c == 0),
                        stop=(c == C - 1),
                    )
            # exp(diag) * mask  (mask kills rows s >= L)
            nc.scalar.activation(diag_exp[:], dg[:], AF.Exp)
            nc.vector.tensor_mul(diag_exp[:], diag_exp[:], maskh[:])

            # ---------------- V projection, interleaved with scores --------
            # Emit projV chunk dd=c between score chunks so the isolated score
            # matmuls' drain latency hides under the projection chains.
            wvts = [wp.tile([128, D], MMDT, name=f"wv{c}", tag="w")
                    for c in range(C)]
            for (off, wd) in blocks(D):
                for c in range(C):
                    nc.scalar.dma_start(
                        wvts[c][:, off:off + wd],
                        w_d["wv"][c * 128:(c + 1) * 128, off:off + wd])
            vt = [vp.tile([128, S], MMDT, name=f"v{c}", tag=f"v{c}")
                  for c in range(C)]

            def projv_chunk(dd):
                for (off, wd) in blocks(S):
                    ps = pp.tile([128, 512], F32, tag="mm")
                    for kk in range(C):
                        nc.tensor.matmul(
                            ps[:, 0:wd],
                            wvts[kk][:, dd * 128:(dd + 1) * 128],
                            xt[kk][:, off:off + wd],
                            start=(kk == 0),
                            stop=(kk == C - 1),
                        )
                    nc.vector.tensor_scalar_add(
                        vt[dd][:, off:off + wd], ps[:, 0:wd], bvt[:, dd:dd + 1])

            # ---------------- scores + exp-accum denominators ----------------
            # Each head's Q is zero-padded to K=128 (zeros align with the other
            # head's K rows and contribute nothing), so score matmuls run at
            # full K=128 rate instead of the slow K=64 fp32r path.
            for c in range(C):
                projv_chunk(c)
                qzs = []
                for p in range(HPC):
                    qz = qzp.tile([128, S], MMDT, name=f"qz{p}", tag="qz")
                    zo = (1 - p) * dk
                    nc.vector.tensor_scalar_mul(qz[zo:zo + dk, :], qt[c][zo:zo + dk, :], 0.0)
                    nc.vector.tensor_copy(
                        qz[p * dk:(p + 1) * dk, :], qt[c][p * dk:(p + 1) * dk, :])
                    qzs.append(qz)
                for i in range(T):
                    N = (i + 1) * 128
                    scs = []
                    for p in range(HPC):
                        sc = pbig.tile([128, min(S, 1024)], F32, name=f"sc{p}", tag="big")
                        for (off, wd) in blocks(N):
                            nc.tensor.matmul(
                                sc[:, off:off + wd],
                                qzs[p][:, i * 128:(i + 1) * 128],
                                kt[c][:, off:off + wd],
                                start=True,
                                stop=True,
                            )
                        scs.append(sc)
                    for p in range(HPC):
                        h = c * HPC + p
                        sc = scs[p]
                        nc.vector.tensor_add(
                            sc[:, i * 128:N], sc[:, i * 128:N], tril[:])
                        nc.scalar.activation(
                            sc[:, 0:N], sc[:, 0:N], AF.Exp,
                            accum_out=dn[i][:, h:h + 1])

            # ---------------- denominators -> a ----------------
            for i in range(T):
                tp = pp.tile([128, 128], F32, tag="mm")
                nc.tensor.transpose(tp[0:H, :], dn[i][:], iden[:])
                nc.vector.tensor_copy(denomT[:, i * 128:(i + 1) * 128], tp[0:H, :])
            nc.vector.reciprocal(arec[:], denomT[:])
            nc.vector.tensor_mul(a_t[:], diag_exp[:], arec[:])

            # ---------------- diagonal weighting of V ----------------
            for c in range(C):
                ab = pbig.tile([128, min(S, 1024)], F32, tag="big")
                for (off, wd) in blocks(S):
                    nc.tensor.matmul(
                        ab[:, off:off + wd],
                        ebc[c][:],
                        a_t[:, off:off + wd],
                        start=True,
                        stop=True,
                    )
                nc.vector.tensor_mul(vt[c][:], vt[c][:], ab[:, 0:S])

            # ---------------- output projection ----------------
            w0ts = [wp.tile([128, D], MMDT, name=f"w0t{c}", tag="w")
                    for c in range(C)]
            for (off, wd) in blocks(D):
                for c in range(C):
                    nc.scalar.dma_start(
                        w0ts[c][:, off:off + wd],
                        w_d["w0"][c * 128:(c + 1) * 128, off:off + wd])
            for m in range(T):
                for (off, wd) in blocks(D):
                    ps = pp.tile([128, 512], F32, tag="mm")
                    for c in range(C):
                        nc.tensor.matmul(
                            ps[:, 0:wd],
                            vt[c][:, m * 128:(m + 1) * 128],
                            w0ts[c][:, off:off + wd],
                            start=(c == 0),
                            stop=(c == C - 1),
                        )
                    ot = outp.tile([128, 512], F32, tag="o")
                    nc.vector.tensor_add(ot[:, 0:wd], ps[:, 0:wd], b0b[:, off:off + wd])
                    nc.sync.dma_start(
                        out_d[m * 128:(m + 1) * 128, off:off + wd], ot[:, 0:wd])

    nc.compile()
    return nc


def _get_nc():
    if "nc" not in _CACHE:
        _CACHE["nc"] = _build(S, D, H)
    return _CACHE["nc"]


def _host_aux(length):
    dk = D // H
    C = D // 128
    aux = {}
    aux["iden"] = np.eye(128, dtype=np.float32)
    tril = np.zeros((128, 128), np.float32)
    tril[np.triu_indices(128, 1)] = NEG
    aux["trilneg"] = tril
    ed = np.zeros((C, 128, H), np.float32)
    ebc = np.zeros((C, H, 128), np.float32)
    for c in range(C):
        for dl in range(128):
            h = (c * 128 + dl) // dk
            ed[c, dl, h] = 1.0
            ebc[c, h, dl] = 1.0
    aux["ed"] = ed
    aux["ebc"] = ebc
    mask = (np.arange(S) < int(length)).astype(np.float32)
    aux["maskh"] = np.tile(mask[None, :], (H, 1))
    return aux


def _in_map(x, wq, bq, wk, bk, wv, bv, w0, b0, length):
    C = D // 128
    inp = {"xt": np.ascontiguousarray(np.asarray(x, np.float32).T)}
    inp["wq"] = np.ascontiguousarray(wq, np.float32)
    inp["wk"] = np.ascontiguousarray(wk, np.float32)
    inp["wv"] = np.ascontiguousarray(wv, np.float32)
    inp["w0"] = np.ascontiguousarray(w0, np.float32)
    inp["bqt"] = np.ascontiguousarray(np.asarray(bq, np.float32).reshape(C, 128).T)
    inp["bkt"] = np.ascontiguousarray(np.asarray(bk, np.float32).reshape(C, 128).T)
    inp["bvt"] = np.ascontiguousarray(np.asarray(bv, np.float32).reshape(C, 128).T)
    inp["b0b"] = np.ascontiguousarray(
        np.tile(np.asarray(b0, np.float32)[None, :], (128, 1)))
    inp.update(_host_aux(length))
    return inp


def _run(inputs, trace=False):
    from concourse.bass_utils import run_bass_kernel_spmd

    batch = np.asarray(inputs["batch"], np.float32)
    lengths = np.asarray(inputs["lengths"])
    nb = batch.shape[0]
    assert batch.shape[1:] == (S, D), batch.shape
    nc = _get_nc()
    in_maps = [
        _in_map(batch[b], inputs["wq"], inputs["bq"], inputs["wk"], inputs["bk"],
                inputs["wv"], inputs["bv"], inputs["w0"], inputs["b0"],
                int(lengths[b]))
        for b in range(nb)
    ]
    res = run_bass_kernel_spmd(nc, in_maps, core_ids=list(range(nb)), trace=trace)
    out = np.stack([r["out"] for r in res.results]).astype(np.float32)
    return out, res


def kernel(**inputs) -> np.ndarray:
    out, _ = _run(inputs, trace=False)
    return out



# revision 10
# speedup vs baseline: 1.0411x; 1.0411x over previous
"""Trainium2 Bass kernel for nn_MultiHeadAttention_85761906966848 (sparse_attention).

The reference module only uses the DIAGONAL of the softmax attention matrix:
    out[b,s,:] = (softmax(masked scores)[s,s] * v[b,s,:]) @ W0 + b0
so no attn @ V matmul is needed — only QK^T row-sums of exp (softmax
denominators), the diagonal q_s.k_s, and the four dense projections.

Sharding: TENSOR-PARALLEL over heads. Core j owns heads (2j, 2j+1):
  * Q/K/V projections restricted to that 128-wide feature slice for ALL
    batches, with the sequence axis trimmed to ceil(L_b/128)*128 valid rows
    (rows past L_b contribute exactly b0, written by the host).
  * Scores / softmax denominators / diagonal weights per local head.
  * O-projection uses the 128-row slice of W0 -> per-core PARTIAL outputs,
    summed on the host (linear combine), which also adds b0.
Uniform SPMD by construction: every core runs the identical program, only the
weight slices in its in_map differ.

All matmuls run in bf16 (1 cycle/column at full PE clock, FWL weight loads);
the causal mask is folded into the score matmul as an extra
(-1e30*I).T @ strict_upper_ones accumulation into the same PSUM tile.
"""

import numpy as np
import ml_dtypes
import concourse.bass as bass
import concourse.bacc as bacc
import concourse.mybir as mybir
from concourse import tile

F32 = mybir.dt.float32
BF16 = mybir.dt.bfloat16
AF = mybir.ActivationFunctionType
AX = mybir.AxisListType
ALU = mybir.AluOpType

B, S, D, H = 8, 1024, 1024, 16
dk = D // H          # 64
HL = 2               # heads per core
NEG = -1.0e30

_CACHE = {}


def blocks(total, width):
    out = []
    off = 0
    while off < total:
        w = min(width, total - off)
        out.append((off, w))
        off += w
    return out


def _build(lts):
    """lts: tuple of per-batch 128-row tile counts, in processing order."""
    NB = len(lts)
    TC = sum(lts)
    SC = TC * 128
    offs = []
    o = 0
    for lt in lts:
        offs.append(o * 128)
        o += lt
    C = D // 128  # 8 contraction chunks for the projections

    nc = bacc.Bacc("TRN2", target_bir_lowering=False, debug=False, num_devices=8)

    xt_d = nc.dram_tensor("xt", [C, 128, SC], BF16, kind="ExternalInput")
    wq_d = nc.dram_tensor("wq", [C, 128, 128], BF16, kind="ExternalInput")
    wk_d = nc.dram_tensor("wk", [C, 128, 128], BF16, kind="ExternalInput")
    wv_d = nc.dram_tensor("wv", [C, 128, 128], BF16, kind="ExternalInput")
    w0_d = nc.dram_tensor("w0", [128, D], BF16, kind="ExternalInput")
    bq_d = nc.dram_tensor("bq", [128, 1], F32, kind="ExternalInput")
    bk_d = nc.dram_tensor("bk", [128, 1], F32, kind="ExternalInput")
    bv_d = nc.dram_tensor("bv", [128, 1], F32, kind="ExternalInput")
    negi_d = nc.dram_tensor("negi", [128, 128], BF16, kind="ExternalInput")
    ub_d = nc.dram_tensor("ub", [128, 128], BF16, kind="ExternalInput")
    idenf_d = nc.dram_tensor("idenf", [128, 128], F32, kind="ExternalInput")
    ones64_d = nc.dram_tensor("ones64", [64, 1], BF16, kind="ExternalInput")
    ind2_d = nc.dram_tensor("ind2", [33, 128], BF16, kind="ExternalInput")
    out_d = nc.dram_tensor("out", [SC, D], BF16, kind="ExternalOutput")

    with tile.TileContext(nc) as tc:
        with (
            tc.tile_pool(name="cp", bufs=1) as cp,
            tc.tile_pool(name="xp", bufs=1) as xp,
            tc.tile_pool(name="qkp", bufs=1) as qkp,
            tc.tile_pool(name="srp", bufs=3) as srp,
            tc.tile_pool(name="smp", bufs=2) as smp,
            tc.tile_pool(name="otp", bufs=3) as otp,
            tc.tile_pool(name="psc", bufs=2, space=bass.MemorySpace.PSUM) as psc,
            tc.tile_pool(name="pmm", bufs=2, space=bass.MemorySpace.PSUM) as pmm,
            tc.tile_pool(name="pab", bufs=2, space=bass.MemorySpace.PSUM) as pab,
        ):
            # ---------------- constants ----------------
            negi = cp.tile([128, 128], BF16, tag="negi")
            nc.sync.dma_start(negi[:], negi_d[:])
            ub = cp.tile([128, 128], BF16, tag="ub")
            nc.sync.dma_start(ub[:], ub_d[:])
            idenf = cp.tile([128, 128], F32, tag="idenf")
            nc.sync.dma_start(idenf[:], idenf_d[:])
            ones64 = cp.tile([64, 1], BF16, tag="ones64")
            nc.sync.dma_start(ones64[:], ones64_d[:])
            ind2 = cp.tile([33, 128], BF16, tag="ind2")
            nc.sync.dma_start(ind2[:], ind2_d[:])
            bq = cp.tile([128, 1], F32, tag="bq")
            nc.sync.dma_start(bq[:], bq_d[:])
            bk = cp.tile([128, 1], F32, tag="bk")
            nc.sync.dma_start(bk[:], bk_d[:])
            bv = cp.tile([128, 1], F32, tag="bv")
            nc.sync.dma_start(bv[:], bv_d[:])

            # weights (K first: K-projection runs first)
            wqs = [cp.tile([128, 128], BF16, name=f"wq{c}", tag=f"wq{c}") for c in range(C)]
            wks = [cp.tile([128, 128], BF16, name=f"wk{c}", tag=f"wk{c}") for c in range(C)]
            wvs = [cp.tile([128, 128], BF16, name=f"wv{c}", tag=f"wv{c}") for c in range(C)]
            for c in range(C):
                nc.scalar.dma_start(wks[c][:], wk_d[c])
            for c in range(C):
                nc.scalar.dma_start(wqs[c][:], wq_d[c])

            # X^T resident, loaded block-major so early columns arrive first
            xt = [xp.tile([128, SC], BF16, name=f"xt{c}", tag=f"xt{c}") for c in range(C)]
            dmae = [nc.sync, nc.gpsimd]
            di = 0
            for (boff, bw) in blocks(SC, 1024):
                for c in range(C):
                    dmae[di % 2].dma_start(xt[c][:, boff:boff + bw],
                                           xt_d[c, :, boff:boff + bw])
                    di += 1
            for c in range(C):
                nc.scalar.dma_start(wvs[c][:], wv_d[c])
            w0s = cp.tile([128, D], BF16, tag="w0s")
            nc.scalar.dma_start(w0s[:], w0_d[:])

            # persistent per-core tensors
            qth = [qkp.tile([64, SC], BF16, name=f"qth{h}", tag=f"qth{h}") for h in range(HL)]
            kth = [qkp.tile([64, SC], BF16, name=f"kth{h}", tag=f"kth{h}") for h in range(HL)]
            vt = qkp.tile([128, SC], BF16, tag="vt")
            a2 = cp.tile([33, SC], BF16, tag="a2")
            nc.vector.memset(a2[:], 0.0)
            dn = [cp.tile([128, 2 * lt], F32, name=f"dn{b}", tag=f"dn{b}")
                  for b, lt in enumerate(lts)]

            # ---------------- K then Q projections ----------------
            def proj_blocks(w_tiles, drain):
                for (boff, bw) in blocks(SC, 512):
                    ps = pmm.tile([128, 512], F32, tag="mm")
                    for kk in range(C):
                        nc.tensor.matmul(ps[:, 0:bw], w_tiles[kk][:],
                                         xt[kk][:, boff:boff + bw],
                                         start=(kk == 0), stop=(kk == C - 1))
                    drain(ps, boff, bw)

            def drain_k(ps, boff, bw):
                for h in range(HL):
                    nc.scalar.activation(kth[h][:, boff:boff + bw],
                                         ps[h * 64:(h + 1) * 64, 0:bw],
                                         AF.Identity, bias=bk[h * 64:(h + 1) * 64, :])

            def drain_q(ps, boff, bw):
                for h in range(HL):
                    nc.vector.tensor_scalar_add(qth[h][:, boff:boff + bw],
                                                ps[h * 64:(h + 1) * 64, 0:bw],
                                                bq[h * 64:(h + 1) * 64, :])

            proj_blocks(wks, drain_k)
            proj_blocks(wqs, drain_q)

            # V-projection blocks emitted lazily between score groups
            vblocks = blocks(SC, 512)
            vstate = {"next": 0}

            def _emit_one_v():
                boff, bw = vblocks[vstate["next"]]
                vstate["next"] += 1
                ps = pmm.tile([128, 512], F32, tag="mm")
                for kk in range(C):
                    nc.tensor.matmul(ps[:, 0:bw], wvs[kk][:],
                                     xt[kk][:, boff:boff + bw],
                                     start=(kk == 0), stop=(kk == C - 1))
                nc.vector.tensor_scalar_add(vt[:, boff:boff + bw], ps[:, 0:bw], bv[:])

            def emit_v_some(n):
                for _ in range(n):
                    if vstate["next"] < len(vblocks):
                        _emit_one_v()

            def emit_v_upto(col_end):
                while (vstate["next"] < len(vblocks)
                       and vblocks[vstate["next"]][0] < col_end):
                    _emit_one_v()

            # ---------------- scores for one batch ----------------
            def emit_scores(b):
                off = offs[b]
                lt = lts[b]
                for i in range(lt):
                    N = (i + 1) * 128
                    for h in range(HL):
                        sc = psc.tile([128, 1024], F32, tag="sc")
                        bl = blocks(N, 512)
                        for bi, (boff, bw) in enumerate(bl):
                            last = (bi == len(bl) - 1)
                            nc.tensor.matmul(sc[:, boff:boff + bw],
                                             qth[h][:, off + i * 128: off + (i + 1) * 128],
                                             kth[h][:, off + boff: off + boff + bw],
                                             start=True, stop=not last)
                        nc.tensor.matmul(sc[:, i * 128:N], negi[:], ub[:],
                                         start=False, stop=True,
                                         skip_group_check=True)
                        scr = srp.tile([128, 1024], BF16, tag="scr")
                        nc.scalar.activation(scr[:, 0:N], sc[:, 0:N], AF.Exp)
                        ci = h * lt + i
                        nc.vector.tensor_reduce(dn[b][:, ci: ci + 1],
                                                scr[:, 0:N], AX.X, ALU.add)
                    emit_v_some(1)

            # ---------------- epilogue for one batch ----------------
            def emit_epilogue(b, oeng):
                off = offs[b]
                lt = lts[b]
                scb = lt * 128
                emit_v_upto(off + scb)
                # diagonal scores: per head, prod = q*k in fp32, split into
                # bf16 hi+lo (error compensation) and column-sum via matmul
                dexpl = [smp.tile([1, 1024], F32, name=f"dexpl{h}", tag=f"dexpl{h}")
                         for h in range(HL)]
                arecl = [smp.tile([1, 1024], F32, name=f"arecl{h}", tag=f"arecl{h}")
                         for h in range(HL)]
                for h in range(HL):
                    for (boff, bw) in blocks(scb, 512):
                        sli = slice(off + boff, off + boff + bw)
                        pr32 = smp.tile([64, 512], F32, tag="pr32")
                        nc.vector.tensor_mul(pr32[:, 0:bw], qth[h][:, sli],
                                             kth[h][:, sli])
                        prh = smp.tile([64, 512], BF16, tag="prh")
                        nc.gpsimd.tensor_copy(prh[:, 0:bw], pr32[:, 0:bw])
                        prl = smp.tile([64, 512], BF16, tag="prl")
                        nc.gpsimd.tensor_sub(prl[:, 0:bw], pr32[:, 0:bw],
                                             prh[:, 0:bw])
                        dg = pmm.tile([1, 512], F32, tag="mm")
                        nc.tensor.matmul(dg[:, 0:bw], ones64[:], prh[:, 0:bw],
                                         start=True, stop=False)
                        nc.tensor.matmul(dg[:, 0:bw], ones64[:], prl[:, 0:bw],
                                         start=False, stop=True)
                        nc.scalar.activation(dexpl[h][:, boff: boff + bw],
                                             dg[:, 0:bw], AF.Exp)
                # reciprocal of denominators, transpose to [2lt, 128]
                rec = smp.tile([128, 16], F32, tag="rec")
                nc.vector.reciprocal(rec[:, 0:2 * lt], dn[b][:])
                tpr = pmm.tile([16, 128], F32, tag="mm")
                nc.tensor.transpose(tpr[0:2 * lt, :], rec[:, 0:2 * lt], idenf[:])
                recT = smp.tile([16, 128], F32, tag="recT")
                nc.vector.tensor_copy(recT[0:2 * lt, :], tpr[0:2 * lt, :])
                # reshape head-grouped rows of recT into per-head lines via DMA
                for h in range(HL):
                    nc.sync.dma_start(arecl[h][:, 0:scb],
                                      recT[h * lt:(h + 1) * lt, :])
                # a2 rows {0,32} hold the two heads' diag attention weights
                for h in range(HL):
                    nc.vector.tensor_mul(a2[32 * h:32 * h + 1, off:off + scb],
                                         dexpl[h][:, 0:scb],
                                         arecl[h][:, 0:scb])
                # broadcast a2 over the 128 local features, weight V in place
                for (boff, bw) in blocks(scb, 512):
                    ab = pab.tile([128, 512], F32, tag="abw")
                    nc.tensor.matmul(ab[:, 0:bw], ind2[:],
                                     a2[:, off + boff: off + boff + bw],
                                     start=True, stop=True)
                    nc.vector.tensor_mul(vt[:, off + boff: off + boff + bw],
                                         vt[:, off + boff: off + boff + bw],
                                         ab[:, 0:bw])
                # output projection per tile, two 512-wide halves
                for i in range(lt):
                    cs = off + i * 128
                    for (ooff, ow) in blocks(D, 512):
                        po = pmm.tile([128, 512], F32, tag="mm")
                        nc.tensor.matmul(po[:, 0:ow], vt[:, cs:cs + 128],
                                         w0s[:, ooff:ooff + ow],
                                         start=True, stop=True)
                        ot = otp.tile([128, 512], BF16, tag="ot")
                        eng = oeng[0]
                        oeng[0] = (oeng[0] + 1) % 2
                        if eng == 0:
                            nc.vector.tensor_copy(ot[:, 0:ow], po[:, 0:ow])
                        else:
                            nc.scalar.copy(ot[:, 0:ow], po[:, 0:ow])
                        nc.sync.dma_start(out_d[cs:cs + 128, ooff:ooff + ow],
                                          ot[:, 0:ow])

            # ---------------- main schedule ----------------
            oeng = [0]
            for b in range(NB):
                emit_scores(b)
                if b > 0:
                    emit_epilogue(b - 1, oeng)
            emit_epilogue(NB - 1, oeng)

    nc.compile()
    return nc


def _get_nc(lts):
    key = tuple(lts)
    if key not in _CACHE:
        _CACHE[key] = _build(key)
    return _CACHE[key]


def _host_consts():
    aux = {}
    negi = np.zeros((128, 128), np.float32)
    np.fill_diagonal(negi, NEG)
    aux["negi"] = negi.astype(ml_dtypes.bfloat16)
    aux["ub"] = np.triu(np.ones((128, 128), np.float32), 1).astype(ml_dtypes.bfloat16)
    aux["idenf"] = np.eye(128, dtype=np.float32)
    aux["ones64"] = np.ones((64, 1), np.float32).astype(ml_dtypes.bfloat16)
    ind2 = np.zeros((33, 128), np.float32)
    ind2[0, 0:64] = 1.0
    ind2[32, 64:128] = 1.0
    aux["ind2"] = ind2.astype(ml_dtypes.bfloat16)
    return aux


def _run(inputs, trace=False):
    from concourse.bass_utils import run_bass_kernel_spmd

    batch = np.asarray(inputs["batch"], np.float32)
    lengths = np.asarray(inputs["lengths"]).astype(np.int64)
    assert batch.shape == (B, S, D), batch.shape
    lt_all = [max(1, int(np.ceil(int(l) / 128.0))) for l in lengths]
    order = sorted(range(B), key=lambda b: -lt_all[b])
    lts = tuple(lt_all[b] for b in order)
    offs = []
    o = 0
    for lt in lts:
        offs.append(o * 128)
        o += lt
    SC = o * 128

    nc = _get_nc(lts)

    # shared across cores
    XT = np.concatenate(
        [batch[order[k]][: lts[k] * 128, :].T for k in range(B)], axis=1)
    xt = np.ascontiguousarray(XT.reshape(D // 128, 128, SC)).astype(ml_dtypes.bfloat16)
    consts = _host_consts()
    wq = np.asarray(inputs["wq"], np.float32)
    wk = np.asarray(inputs["wk"], np.float32)
    wv = np.asarray(inputs["wv"], np.float32)
    w0 = np.asarray(inputs["w0"], np.float32)
    bqf = np.asarray(inputs["bq"], np.float32)
    bkf = np.asarray(inputs["bk"], np.float32)
    bvf = np.asarray(inputs["bv"], np.float32)

    in_maps = []
    for j in range(8):
        sl = slice(j * 128, (j + 1) * 128)
        im = dict(consts)
        im["xt"] = xt
        im["wq"] = np.ascontiguousarray(
            wq[:, sl].reshape(8, 128, 128)).astype(ml_dtypes.bfloat16)
        im["wk"] = np.ascontiguousarray(
            wk[:, sl].reshape(8, 128, 128)).astype(ml_dtypes.bfloat16)
        im["wv"] = np.ascontiguousarray(
            wv[:, sl].reshape(8, 128, 128)).astype(ml_dtypes.bfloat16)
        im["w0"] = np.ascontiguousarray(w0[sl, :]).astype(ml_dtypes.bfloat16)
        im["bq"] = np.ascontiguousarray(bqf[sl].reshape(128, 1))
        im["bk"] = np.ascontiguousarray(bkf[sl].reshape(128, 1))
        im["bv"] = np.ascontiguousarray(bvf[sl].reshape(128, 1))
        in_maps.append(im)

    res = run_bass_kernel_spmd(nc, in_maps, core_ids=list(range(8)), trace=trace)

    acc = np.zeros((SC, D), np.float32)
    for r in res.results:
        acc += np.asarray(r["out"]).astype(np.float32)
    b0 = np.asarray(inputs["b0"], np.float32)
    out = np.empty((B, S, D), np.float32)
    out[:] = b0[None, None, :]
    for k in range(B):
        b = order[k]
        L = int(lengths[b])
        out[b, :L, :] += acc[offs[k]: offs[k] + L, :]
    return out, res


def kernel(**inputs) -> np.ndarray:
    out, _ = _run(inputs, trace=False)
    return out


# revision 12
# speedup vs baseline: 1.0587x; 1.0169x over previous
"""Trainium2 Bass kernel for nn_MultiHeadAttention_85761906966848 (sparse_attention).

The reference module only uses the DIAGONAL of the softmax attention matrix:
    out[b,s,:] = (softmax(masked scores)[s,s] * v[b,s,:]) @ W0 + b0
so no attn @ V matmul is needed — only QK^T row-sums of exp (softmax
denominators), the diagonal q_s.k_s, and the four dense projections.

Sharding: TENSOR-PARALLEL over heads. Core j owns heads (2j, 2j+1):
  * Q/K/V projections restricted to that 128-wide feature slice for ALL
    batches, with the sequence axis trimmed to ceil(L_b/128)*128 valid rows
    (rows past L_b contribute exactly b0, written by the host).
  * Scores / softmax denominators / diagonal weights per local head.
  * O-projection uses the 128-row slice of W0 -> per-core PARTIAL outputs,
    summed on the host (linear combine), which also adds b0.
Uniform SPMD by construction: every core runs the identical program, only the
weight slices in its in_map differ.

All matmuls run in bf16; the causal mask is folded into the score matmul as an
extra (-1e30*I).T @ strict_upper_ones accumulation into the same PSUM tile;
softmax denominators come from the scalar engine's activation accumulator.
Epilogue work for batch b-1 is woven between the score tiles of batch b so the
in-order tensor queue never stalls on cross-engine dependencies.
"""

import numpy as np
import ml_dtypes
import concourse.bass as bass
import concourse.bacc as bacc
import concourse.mybir as mybir
from concourse import tile

F32 = mybir.dt.float32
BF16 = mybir.dt.bfloat16
AF = mybir.ActivationFunctionType

B, S, D, H = 8, 1024, 1024, 16
dk = D // H          # 64
HL = 2               # heads per core
NEG = -1.0e30

_CACHE = {}


def blocks(total, width):
    out = []
    off = 0
    while off < total:
        w = min(width, total - off)
        out.append((off, w))
        off += w
    return out


def _build(lts):
    """lts: tuple of per-batch 128-row tile counts, in processing order."""
    NB = len(lts)
    TC = sum(lts)
    SC = TC * 128
    offs = []
    o = 0
    for lt in lts:
        offs.append(o * 128)
        o += lt
    C = D // 128  # 8 contraction chunks for the projections

    nc = bacc.Bacc("TRN2", target_bir_lowering=False, debug=False, num_devices=8)

    xt_d = nc.dram_tensor("xt", [C, 128, SC], BF16, kind="ExternalInput")
    wq_d = nc.dram_tensor("wq", [C, 128, 128], BF16, kind="ExternalInput")
    wk_d = nc.dram_tensor("wk", [C, 128, 128], BF16, kind="ExternalInput")
    wv_d = nc.dram_tensor("wv", [C, 128, 128], BF16, kind="ExternalInput")
    w0_d = nc.dram_tensor("w0", [128, D], BF16, kind="ExternalInput")
    bq_d = nc.dram_tensor("bq", [128, 1], F32, kind="ExternalInput")
    bk_d = nc.dram_tensor("bk", [128, 1], F32, kind="ExternalInput")
    bv_d = nc.dram_tensor("bv", [128, 1], F32, kind="ExternalInput")
    negi_d = nc.dram_tensor("negi", [128, 128], BF16, kind="ExternalInput")
    ub_d = nc.dram_tensor("ub", [128, 128], BF16, kind="ExternalInput")
    idenf_d = nc.dram_tensor("idenf", [128, 128], F32, kind="ExternalInput")
    ones64_d = nc.dram_tensor("ones64", [64, 1], BF16, kind="ExternalInput")
    ind2_d = nc.dram_tensor("ind2", [33, 128], BF16, kind="ExternalInput")
    out_d = nc.dram_tensor("out", [SC, D], BF16, kind="ExternalOutput")

    with tile.TileContext(nc) as tc:
        with (
            tc.tile_pool(name="cp", bufs=1) as cp,
            tc.tile_pool(name="xp", bufs=1) as xp,
            tc.tile_pool(name="qkp", bufs=1) as qkp,
            tc.tile_pool(name="srp", bufs=3) as srp,
            tc.tile_pool(name="prp", bufs=3) as prp,
            tc.tile_pool(name="dlp", bufs=2) as dlp,
            tc.tile_pool(name="otp", bufs=3) as otp,
            tc.tile_pool(name="psc", bufs=2, space=bass.MemorySpace.PSUM) as psc,
            tc.tile_pool(name="pmm", bufs=2, space=bass.MemorySpace.PSUM) as pmm,
            tc.tile_pool(name="pab", bufs=2, space=bass.MemorySpace.PSUM) as pab,
        ):
            # ---------------- constants (sync queue) ----------------
            negi = cp.tile([128, 128], BF16, tag="negi")
            nc.sync.dma_start(negi[:], negi_d[:])
            ub = cp.tile([128, 128], BF16, tag="ub")
            nc.sync.dma_start(ub[:], ub_d[:])
            idenf = cp.tile([128, 128], F32, tag="idenf")
            nc.sync.dma_start(idenf[:], idenf_d[:])
            ones64 = cp.tile([64, 1], BF16, tag="ones64")
            nc.sync.dma_start(ones64[:], ones64_d[:])
            ind2 = cp.tile([33, 128], BF16, tag="ind2")
            nc.sync.dma_start(ind2[:], ind2_d[:])
            bq = cp.tile([128, 1], F32, tag="bq")
            nc.sync.dma_start(bq[:], bq_d[:])
            bk = cp.tile([128, 1], F32, tag="bk")
            nc.sync.dma_start(bk[:], bk_d[:])
            bv = cp.tile([128, 1], F32, tag="bv")
            nc.sync.dma_start(bv[:], bv_d[:])

            # weights on scalar queue first (small, needed first)
            wqs = [cp.tile([128, 128], BF16, name=f"wq{c}", tag=f"wq{c}") for c in range(C)]
            wks = [cp.tile([128, 128], BF16, name=f"wk{c}", tag=f"wk{c}") for c in range(C)]
            wvs = [cp.tile([128, 128], BF16, name=f"wv{c}", tag=f"wv{c}") for c in range(C)]
            for c in range(C):
                nc.scalar.dma_start(wks[c][:], wk_d[c, :, :])
            for c in range(C):
                nc.scalar.dma_start(wqs[c][:], wq_d[c, :, :])

            # X^T resident, block-major on the two HW DMA queues
            xt = [xp.tile([128, SC], BF16, name=f"xt{c}", tag=f"xt{c}") for c in range(C)]
            di = 0
            for (boff, bw) in blocks(SC, 1024):
                for c in range(C):
                    eng = nc.sync if di % 2 == 0 else nc.scalar
                    eng.dma_start(xt[c][:, boff:boff + bw],
                                  xt_d[c, :, boff:boff + bw])
                    di += 1
            for c in range(C):
                nc.scalar.dma_start(wvs[c][:], wv_d[c, :, :])
            w0s = cp.tile([128, D], BF16, tag="w0s")
            nc.scalar.dma_start(w0s[:], w0_d[:])

            # persistent per-core tensors
            qth = [qkp.tile([64, SC], BF16, name=f"qth{h}", tag=f"qth{h}") for h in range(HL)]
            kth = [qkp.tile([64, SC], BF16, name=f"kth{h}", tag=f"kth{h}") for h in range(HL)]
            vt = qkp.tile([128, SC], BF16, tag="vt")
            a2 = cp.tile([33, SC], BF16, tag="a2")
            nc.vector.memset(a2[:], 0.0)
            dn = [cp.tile([128, 2 * lt], F32, name=f"dn{b}", tag=f"dn{b}")
                  for b, lt in enumerate(lts)]

            # ---------------- K then Q projections ----------------
            def proj_blocks(w_tiles, drain):
                for (boff, bw) in blocks(SC, 512):
                    ps = pmm.tile([128, 512], F32, tag="mm")
                    for kk in range(C):
                        nc.tensor.matmul(ps[:, 0:bw], w_tiles[kk][:],
                                         xt[kk][:, boff:boff + bw],
                                         start=(kk == 0), stop=(kk == C - 1))
                    drain(ps, boff, bw)

            def drain_k(ps, boff, bw):
                nc.scalar.activation(kth[0][:, boff:boff + bw], ps[0:64, 0:bw],
                                     AF.Identity, bias=bk[0:64, :])
                nc.vector.tensor_scalar_add(kth[1][:, boff:boff + bw],
                                            ps[64:128, 0:bw], bk[64:128, :])

            def drain_q(ps, boff, bw):
                nc.vector.tensor_scalar_add(qth[0][:, boff:boff + bw],
                                            ps[0:64, 0:bw], bq[0:64, :])
                nc.scalar.activation(qth[1][:, boff:boff + bw], ps[64:128, 0:bw],
                                     AF.Identity, bias=bq[64:128, :])

            proj_blocks(wks, drain_k)
            proj_blocks(wqs, drain_q)

            # V-projection blocks emitted lazily between score groups
            vblocks = blocks(SC, 512)
            vstate = {"next": 0}

            def _emit_one_v():
                boff, bw = vblocks[vstate["next"]]
                vstate["next"] += 1
                ps = pmm.tile([128, 512], F32, tag="mm")
                for kk in range(C):
                    nc.tensor.matmul(ps[:, 0:bw], wvs[kk][:],
                                     xt[kk][:, boff:boff + bw],
                                     start=(kk == 0), stop=(kk == C - 1))
                nc.vector.tensor_scalar_add(vt[:, boff:boff + bw], ps[:, 0:bw], bv[:])

            def emit_v_some(n):
                for _ in range(n):
                    if vstate["next"] < len(vblocks):
                        _emit_one_v()

            def emit_v_upto(col_end):
                while (vstate["next"] < len(vblocks)
                       and vblocks[vstate["next"]][0] < col_end):
                    _emit_one_v()

            # ------------- epilogue of batch e as a unit queue -------------
            # prod (q*k hi/lo) for batch e is emitted during scores(e) itself;
            # the remaining units are pumped between score tiles of batch e+1.
            prod_tiles = {}

            def emit_prod(e, i):
                """q*k product for tile i of batch e (vector+gpsimd only).
                prh/prl live until the weave of batch e+1 consumes them, so
                they get per-(tile,head) tags, double-buffered across batches."""
                off = offs[e]
                sli = slice(off + i * 128, off + (i + 1) * 128)
                pair = []
                for h in range(HL):
                    pr32 = prp.tile([64, 128], F32, tag="pr32")
                    nc.vector.tensor_mul(pr32[:], qth[h][:, sli], kth[h][:, sli])
                    prh = prp.tile([64, 128], BF16, name=f"prh{i}_{h}",
                                   tag=f"prh{i}_{h}", bufs=2)
                    nc.gpsimd.tensor_copy(prh[:], pr32[:])
                    prl = prp.tile([64, 128], BF16, name=f"prl{i}_{h}",
                                   tag=f"prl{i}_{h}", bufs=2)
                    nc.gpsimd.tensor_sub(prl[:], pr32[:], prh[:])
                    pair.append((prh, prl))
                prod_tiles[(e, i)] = pair

            def epilogue_units(e, oeng):
                off = offs[e]
                lt = lts[e]
                scb = lt * 128
                st = {}

                def u_recip():
                    emit_v_upto(off + scb)
                    rec = prp.tile([128, 16], F32, tag="rec")
                    nc.vector.reciprocal(rec[:, 0:2 * lt], dn[e][:])
                    tpr = pmm.tile([16, 128], F32, tag="mm")
                    nc.tensor.transpose(tpr[0:2 * lt, :], rec[:, 0:2 * lt], idenf[:])
                    recT = prp.tile([16, 128], F32, tag="recT")
                    nc.vector.tensor_copy(recT[0:2 * lt, :], tpr[0:2 * lt, :])
                    st["arecl"] = [dlp.tile([1, 1024], F32, name=f"arecl{h}",
                                            tag=f"arecl{h}") for h in range(HL)]
                    for h in range(HL):
                        nc.sync.dma_start(st["arecl"][h][:, 0:scb],
                                          recT[h * lt:(h + 1) * lt, :])
                yield u_recip

                def u_diag():
                    st["dexpl"] = [dlp.tile([1, 1024], F32, name=f"dexpl{h}",
                                            tag=f"dexpl{h}") for h in range(HL)]
                    for h in range(HL):
                        for i in range(lt):
                            prh, prl = prod_tiles[(e, i)][h]
                            dg = pmm.tile([1, 512], F32, tag="mm")
                            nc.tensor.matmul(dg[:, 0:128], ones64[:], prh[:],
                                             start=True, stop=False)
                            nc.tensor.matmul(dg[:, 0:128], ones64[:], prl[:],
                                             start=False, stop=True)
                            nc.scalar.activation(
                                st["dexpl"][h][:, i * 128:(i + 1) * 128],
                                dg[:, 0:128], AF.Exp)
                    for i in range(lt):
                        del prod_tiles[(e, i)]
                yield u_diag

                def u_a2():
                    for h in range(HL):
                        nc.vector.tensor_mul(a2[32 * h:32 * h + 1, off:off + scb],
                                             st["dexpl"][h][:, 0:scb],
                                             st["arecl"][h][:, 0:scb])
                yield u_a2

                for (boff, bw) in blocks(scb, 512):
                    def u_ab(boff=boff, bw=bw):
                        ab = pab.tile([128, 512], F32, tag="abw")
                        nc.tensor.matmul(ab[:, 0:bw], ind2[:],
                                         a2[:, off + boff: off + boff + bw],
                                         start=True, stop=True)
                        nc.vector.tensor_mul(vt[:, off + boff: off + boff + bw],
                                             vt[:, off + boff: off + boff + bw],
                                             ab[:, 0:bw])
                    yield u_ab

                for i in range(lt):
                    def u_out(i=i):
                        cs = off + i * 128
                        ot = otp.tile([128, 1024], BF16, tag="ot")
                        for oi, (ooff, ow) in enumerate(blocks(D, 512)):
                            po = pmm.tile([128, 512], F32, tag="mm")
                            nc.tensor.matmul(po[:, 0:ow], vt[:, cs:cs + 128],
                                             w0s[:, ooff:ooff + ow],
                                             start=True, stop=True)
                            eng = oeng[0]
                            oeng[0] = (oeng[0] + 1) % 2
                            if eng == 0:
                                nc.vector.tensor_copy(ot[:, ooff:ooff + ow],
                                                      po[:, 0:ow])
                            else:
                                nc.scalar.copy(ot[:, ooff:ooff + ow], po[:, 0:ow])
                        nc.sync.dma_start(out_d[cs:cs + 128, :], ot[:])
                    yield u_out

            # ---------------- scores with woven epilogue ----------------
            def emit_scores(b, pending):
                off = offs[b]
                lt = lts[b]
                for i in range(lt):
                    N = (i + 1) * 128
                    for h in range(HL):
                        sc = psc.tile([128, 1024], F32, tag="sc")
                        bl = blocks(N, 512)
                        for bi, (boff, bw) in enumerate(bl):
                            last = (bi == len(bl) - 1)
                            nc.tensor.matmul(sc[:, boff:boff + bw],
                                             qth[h][:, off + i * 128: off + (i + 1) * 128],
                                             kth[h][:, off + boff: off + boff + bw],
                                             start=True, stop=not last)
                        nc.tensor.matmul(sc[:, i * 128:N], negi[:], ub[:],
                                         start=False, stop=True,
                                         skip_group_check=True)
                        scr = srp.tile([128, 1024], BF16, tag="scr")
                        ci = h * lt + i
                        nc.scalar.activation(scr[:, 0:N], sc[:, 0:N], AF.Exp,
                                             accum_out=dn[b][:, ci:ci + 1])
                    emit_prod(b, i)
                    emit_v_some(1)
                    # pump up to two pending epilogue units of batch b-1
                    for _ in range(2):
                        if pending:
                            pending.pop(0)()

            # ---------------- main schedule ----------------
            oeng = [0]
            pending = []
            for b in range(NB):
                emit_scores(b, pending)
                while pending:
                    pending.pop(0)()
                pending = list(epilogue_units(b, oeng))
            while pending:
                pending.pop(0)()

    nc.compile()
    return nc


def _get_nc(lts):
    key = tuple(lts)
    if key not in _CACHE:
        _CACHE[key] = _build(key)
    return _CACHE[key]


def _host_consts():
    aux = {}
    negi = np.zeros((128, 128), np.float32)
    np.fill_diagonal(negi, NEG)
    aux["negi"] = negi.astype(ml_dtypes.bfloat16)
    aux["ub"] = np.triu(np.ones((128, 128), np.float32), 1).astype(ml_dtypes.bfloat16)
    aux["idenf"] = np.eye(128, dtype=np.float32)
    aux["ones64"] = np.ones((64, 1), np.float32).astype(ml_dtypes.bfloat16)
    ind2 = np.zeros((33, 128), np.float32)
    ind2[0, 0:64] = 1.0
    ind2[32, 64:128] = 1.0
    aux["ind2"] = ind2.astype(ml_dtypes.bfloat16)
    return aux


def _run(inputs, trace=False):
    from concourse.bass_utils import run_bass_kernel_spmd

    batch = np.asarray(inputs["batch"], np.float32)
    lengths = np.asarray(inputs["lengths"]).astype(np.int64)
    assert batch.shape == (B, S, D), batch.shape
    lt_all = [max(1, int(np.ceil(int(l) / 128.0))) for l in lengths]
    order = sorted(range(B), key=lambda b: -lt_all[b])
    lts = tuple(lt_all[b] for b in order)
    offs = []
    o = 0
    for lt in lts:
        offs.append(o * 128)
        o += lt
    SC = o * 128

    nc = _get_nc(lts)

    # shared across cores
    XT = np.concatenate(
        [batch[order[k]][: lts[k] * 128, :].T for k in range(B)], axis=1)
    xt = np.ascontiguousarray(XT.reshape(D // 128, 128, SC)).astype(ml_dtypes.bfloat16)
    consts = _host_consts()
    wq = np.asarray(inputs["wq"], np.float32)
    wk = np.asarray(inputs["wk"], np.float32)
    wv = np.asarray(inputs["wv"], np.float32)
    w0 = np.asarray(inputs["w0"], np.float32)
    bqf = np.asarray(inputs["bq"], np.float32)
    bkf = np.asarray(inputs["bk"], np.float32)
    bvf = np.asarray(inputs["bv"], np.float32)

    in_maps = []
    for j in range(8):
        sl = slice(j * 128, (j + 1) * 128)
        im = dict(consts)
        im["xt"] = xt
        im["wq"] = np.ascontiguousarray(
            wq[:, sl].reshape(8, 128, 128)).astype(ml_dtypes.bfloat16)
        im["wk"] = np.ascontiguousarray(
            wk[:, sl].reshape(8, 128, 128)).astype(ml_dtypes.bfloat16)
        im["wv"] = np.ascontiguousarray(
            wv[:, sl].reshape(8, 128, 128)).astype(ml_dtypes.bfloat16)
        im["w0"] = np.ascontiguousarray(w0[sl, :]).astype(ml_dtypes.bfloat16)
        im["bq"] = np.ascontiguousarray(bqf[sl].reshape(128, 1))
        im["bk"] = np.ascontiguousarray(bkf[sl].reshape(128, 1))
        im["bv"] = np.ascontiguousarray(bvf[sl].reshape(128, 1))
        in_maps.append(im)

    res = run_bass_kernel_spmd(nc, in_maps, core_ids=list(range(8)), trace=trace)

    acc = np.zeros((SC, D), np.float32)
    for r in res.results:
        acc += np.asarray(r["out"]).astype(np.float32)
    b0 = np.asarray(inputs["b0"], np.float32)
    out = np.empty((B, S, D), np.float32)
    out[:] = b0[None, None, :]
    for k in range(B):
        b = order[k]
        L = int(lengths[b])
        out[b, :L, :] += acc[offs[k]: offs[k] + L, :]
    return out, res


def kernel(**inputs) -> np.ndarray:
    out, _ = _run(inputs, trace=False)
    return out


# revision 17
# speedup vs baseline: 1.0687x; 1.0094x over previous
"""Trainium2 Bass kernel for nn_MultiHeadAttention_85761906966848 (sparse_attention).

The reference module only uses the DIAGONAL of the softmax attention matrix:
    out[b,s,:] = (softmax(masked scores)[s,s] * v[b,s,:]) @ W0 + b0
so no attn @ V matmul is needed — only QK^T row-sums of exp (softmax
denominators), the diagonal q_s.k_s, and the four dense projections.

Sharding: TENSOR-PARALLEL over heads. Core j owns heads (2j, 2j+1):
  * Q/K/V projections restricted to that 128-wide feature slice for ALL
    batches, sequence axis trimmed to ceil(L_b/128)*128 valid rows.
  * Scores / softmax denominators / diagonal weights per local head.
  * O-projection uses the 128-row slice of W0 -> per-core PARTIAL outputs,
    summed on the host (linear combine), which also adds b0.
Uniform SPMD by construction; only the weight slices per in_map differ.

Matmuls in bf16; causal mask folded into the score matmul as an extra
(-1e30*I).T @ strict_upper_ones PSUM accumulation. Score tiles of width<=512
pack BOTH heads into one PSUM tile: one exp + one strided 3-D reduce per
pair, halving the scalar-engine op count on the critical softmax path.
Epilogue work of batch b-1 is woven between score tiles of batch b.
"""

import numpy as np
import ml_dtypes
import concourse.bass as bass
import concourse.bacc as bacc
import concourse.mybir as mybir
from concourse import tile

F32 = mybir.dt.float32
BF16 = mybir.dt.bfloat16
AF = mybir.ActivationFunctionType
AX = mybir.AxisListType
ALU = mybir.AluOpType

B, S, D, H = 8, 1024, 1024, 16
dk = D // H
HL = 2
NEG = -1.0e30

_CACHE = {}


def blocks(total, width):
    out = []
    off = 0
    while off < total:
        w = min(width, total - off)
        out.append((off, w))
        off += w
    return out


def _build(lts):
    """lts: tuple of per-batch 128-row tile counts, in processing order."""
    NB = len(lts)
    TC = sum(lts)
    SC = TC * 128
    offs = []
    o = 0
    for lt in lts:
        offs.append(o * 128)
        o += lt
    C = D // 128

    nc = bacc.Bacc("TRN2", target_bir_lowering=False, debug=False, num_devices=8)

    xt_d = nc.dram_tensor("xt", [C, 128, SC], BF16, kind="ExternalInput")
    wq_d = nc.dram_tensor("wq", [C, 128, 128], BF16, kind="ExternalInput")
    wk_d = nc.dram_tensor("wk", [C, 128, 128], BF16, kind="ExternalInput")
    wv_d = nc.dram_tensor("wv", [C, 128, 128], BF16, kind="ExternalInput")
    w0_d = nc.dram_tensor("w0", [128, D], BF16, kind="ExternalInput")
    bq_d = nc.dram_tensor("bq", [128, 1], F32, kind="ExternalInput")
    bk_d = nc.dram_tensor("bk", [128, 1], F32, kind="ExternalInput")
    bv_d = nc.dram_tensor("bv", [128, 1], F32, kind="ExternalInput")
    negi_d = nc.dram_tensor("negi", [128, 128], BF16, kind="ExternalInput")
    ub_d = nc.dram_tensor("ub", [128, 128], BF16, kind="ExternalInput")
    idenf_d = nc.dram_tensor("idenf", [128, 128], F32, kind="ExternalInput")
    ones64_d = nc.dram_tensor("ones64", [64, 1], BF16, kind="ExternalInput")
    ind2_d = nc.dram_tensor("ind2", [33, 128], BF16, kind="ExternalInput")
    out_d = nc.dram_tensor("out", [SC, D], BF16, kind="ExternalOutput")

    with tile.TileContext(nc) as tc:
        with (
            tc.tile_pool(name="cp", bufs=1) as cp,
            tc.tile_pool(name="xp", bufs=1) as xp,
            tc.tile_pool(name="qkp", bufs=1) as qkp,
            tc.tile_pool(name="srp", bufs=3) as srp,
            tc.tile_pool(name="prp", bufs=3) as prp,
            tc.tile_pool(name="dlp", bufs=2) as dlp,
            tc.tile_pool(name="otp", bufs=3) as otp,
            tc.tile_pool(name="psc", bufs=3, space=bass.MemorySpace.PSUM) as psc,
            tc.tile_pool(name="pmm", bufs=2, space=bass.MemorySpace.PSUM) as pmm,
        ):
            # ---------------- constants (sync queue) ----------------
            negi = cp.tile([128, 128], BF16, tag="negi")
            nc.sync.dma_start(negi[:], negi_d[:])
            ub = cp.tile([128, 128], BF16, tag="ub")
            nc.sync.dma_start(ub[:], ub_d[:])
            idenf = cp.tile([128, 128], F32, tag="idenf")
            nc.sync.dma_start(idenf[:], idenf_d[:])
            ones64 = cp.tile([64, 1], BF16, tag="ones64")
            nc.sync.dma_start(ones64[:], ones64_d[:])
            ind2 = cp.tile([33, 128], BF16, tag="ind2")
            nc.sync.dma_start(ind2[:], ind2_d[:])
            bq = cp.tile([128, 1], F32, tag="bq")
            nc.sync.dma_start(bq[:], bq_d[:])
            bk = cp.tile([128, 1], F32, tag="bk")
            nc.sync.dma_start(bk[:], bk_d[:])
            bv = cp.tile([128, 1], F32, tag="bv")
            nc.sync.dma_start(bv[:], bv_d[:])

            # weights on scalar queue first (small, needed first)
            wqs = [cp.tile([128, 128], BF16, name=f"wq{c}", tag=f"wq{c}") for c in range(C)]
            wks = [cp.tile([128, 128], BF16, name=f"wk{c}", tag=f"wk{c}") for c in range(C)]
            wvs = [cp.tile([128, 128], BF16, name=f"wv{c}", tag=f"wv{c}") for c in range(C)]
            for c in range(C):
                nc.scalar.dma_start(wks[c][:], wk_d[c, :, :])
            for c in range(C):
                nc.scalar.dma_start(wqs[c][:], wq_d[c, :, :])

            # X^T resident: narrow first block for a fast start, wide after
            xt = [xp.tile([128, SC], BF16, name=f"xt{c}", tag=f"xt{c}") for c in range(C)]
            xblocks = [(0, min(512, SC))] + blocks(SC - min(512, SC), 2048)
            xblocks = [(0, xblocks[0][1])] + [(512 + o, w) for (o, w) in xblocks[1:]]
            di = 0
            for (boff, bw) in xblocks:
                for c in range(C):
                    eng = nc.sync if di % 2 == 0 else nc.scalar
                    eng.dma_start(xt[c][:, boff:boff + bw],
                                  xt_d[c, :, boff:boff + bw])
                    di += 1
            for c in range(C):
                nc.scalar.dma_start(wvs[c][:], wv_d[c, :, :])
            w0s = cp.tile([128, D], BF16, tag="w0s")
            nc.scalar.dma_start(w0s[:], w0_d[:])

            # persistent per-core tensors
            qth = [qkp.tile([64, SC], BF16, name=f"qth{h}", tag=f"qth{h}") for h in range(HL)]
            kth = [qkp.tile([64, SC], BF16, name=f"kth{h}", tag=f"kth{h}") for h in range(HL)]
            vt = qkp.tile([128, SC], BF16, tag="vt")
            a2 = cp.tile([33, SC], BF16, tag="a2")
            nc.vector.memset(a2[:], 0.0)
            dn = [cp.tile([128, 2 * lt], F32, name=f"dn{b}", tag=f"dn{b}")
                  for b, lt in enumerate(lts)]

            # ---------------- K then Q projections ----------------
            def proj_blocks(w_tiles, drain):
                for (boff, bw) in blocks(SC, 512):
                    ps = pmm.tile([128, 512], F32, tag="mm")
                    for kk in range(C):
                        nc.tensor.matmul(ps[:, 0:bw], w_tiles[kk][:],
                                         xt[kk][:, boff:boff + bw],
                                         start=(kk == 0), stop=(kk == C - 1))
                    drain(ps, boff, bw)

            def drain_k(ps, boff, bw):
                nc.scalar.activation(kth[0][:, boff:boff + bw], ps[0:64, 0:bw],
                                     AF.Identity, bias=bk[0:64, :])
                nc.vector.tensor_scalar_add(kth[1][:, boff:boff + bw],
                                            ps[64:128, 0:bw], bk[64:128, :])

            def drain_q(ps, boff, bw):
                nc.vector.tensor_scalar_add(qth[0][:, boff:boff + bw],
                                            ps[0:64, 0:bw], bq[0:64, :])
                nc.scalar.activation(qth[1][:, boff:boff + bw], ps[64:128, 0:bw],
                                     AF.Identity, bias=bq[64:128, :])

            proj_blocks(wks, drain_k)
            proj_blocks(wqs, drain_q)

            # V-projection blocks emitted lazily between score groups
            vblocks = blocks(SC, 512)
            vstate = {"next": 0}

            def _emit_one_v():
                boff, bw = vblocks[vstate["next"]]
                vstate["next"] += 1
                ps = pmm.tile([128, 512], F32, tag="mm")
                for kk in range(C):
                    nc.tensor.matmul(ps[:, 0:bw], wvs[kk][:],
                                     xt[kk][:, boff:boff + bw],
                                     start=(kk == 0), stop=(kk == C - 1))
                nc.vector.tensor_scalar_add(vt[:, boff:boff + bw], ps[:, 0:bw], bv[:])

            def emit_v_some(n):
                for _ in range(n):
                    if vstate["next"] < len(vblocks):
                        _emit_one_v()

            def emit_v_upto(col_end):
                while (vstate["next"] < len(vblocks)
                       and vblocks[vstate["next"]][0] < col_end):
                    _emit_one_v()

            # ---------------- diag products (q*k hi/lo), per 512 block -----
            prod_tiles = {}

            def emit_prod(e):
                off = offs[e]
                scb = lts[e] * 128
                for h in range(HL):
                    for bi, (boff, bw) in enumerate(blocks(scb, 512)):
                        sli = slice(off + boff, off + boff + bw)
                        pr32 = prp.tile([64, 512], F32, tag="pr32")
                        nc.vector.tensor_mul(pr32[:, 0:bw], qth[h][:, sli],
                                             kth[h][:, sli])
                        prh = prp.tile([64, 512], BF16, name=f"prh{h}_{bi}",
                                       tag=f"prh{h}_{bi}", bufs=1)
                        nc.gpsimd.tensor_copy(prh[:, 0:bw], pr32[:, 0:bw])
                        prl = prp.tile([64, 512], BF16, name=f"prl{h}_{bi}",
                                       tag=f"prl{h}_{bi}", bufs=1)
                        nc.gpsimd.tensor_sub(prl[:, 0:bw], pr32[:, 0:bw],
                                             prh[:, 0:bw])
                        prod_tiles[(e, h, bi)] = (prh, prl, bw)

            # ------------- epilogue of batch e as a unit queue -------------
            def epilogue_units(e, oeng):
                off = offs[e]
                lt = lts[e]
                scb = lt * 128
                st = {}

                def u_recip():
                    emit_v_upto(off + scb)
                    rec = prp.tile([128, 16], F32, tag="rec")
                    nc.vector.reciprocal(rec[:, 0:2 * lt], dn[e][:])
                    tpr = pmm.tile([16, 128], F32, tag="mm")
                    nc.tensor.transpose(tpr[0:2 * lt, :], rec[:, 0:2 * lt], idenf[:])
                    recT = prp.tile([16, 128], BF16, tag="recT")
                    nc.vector.tensor_copy(recT[0:2 * lt, :], tpr[0:2 * lt, :])
                    st["arecl"] = [dlp.tile([1, 1024], BF16, name=f"arecl{h}",
                                            tag=f"arecl{h}") for h in range(HL)]
                    for h in range(HL):
                        nc.sync.dma_start(st["arecl"][h][:, 0:scb],
                                          recT[h:2 * lt:2, :])
                yield u_recip

                def u_diag():
                    st["dexpl"] = [dlp.tile([1, 1024], BF16, name=f"dexpl{h}",
                                            tag=f"dexpl{h}") for h in range(HL)]
                    for h in range(HL):
                        for bi, (boff, bw) in enumerate(blocks(scb, 512)):
                            prh, prl, _ = prod_tiles.pop((e, h, bi))
                            dg = pmm.tile([1, 512], F32, tag="mm")
                            nc.tensor.matmul(dg[:, 0:bw], ones64[:], prh[:, 0:bw],
                                             start=True, stop=False)
                            nc.tensor.matmul(dg[:, 0:bw], ones64[:], prl[:, 0:bw],
                                             start=False, stop=True)
                            nc.scalar.activation(st["dexpl"][h][:, boff:boff + bw],
                                                 dg[:, 0:bw], AF.Exp)
                yield u_diag

                def u_a2():
                    for h in range(HL):
                        nc.vector.tensor_mul(a2[32 * h:32 * h + 1, off:off + scb],
                                             st["dexpl"][h][:, 0:scb],
                                             st["arecl"][h][:, 0:scb])
                yield u_a2

                for (boff, bw) in blocks(scb, 512):
                    def u_ab(boff=boff, bw=bw):
                        ab = pmm.tile([128, 512], F32, tag="mm")
                        nc.tensor.matmul(ab[:, 0:bw], ind2[:],
                                         a2[:, off + boff: off + boff + bw],
                                         start=True, stop=True)
                        nc.vector.tensor_mul(vt[:, off + boff: off + boff + bw],
                                             vt[:, off + boff: off + boff + bw],
                                             ab[:, 0:bw])
                    yield u_ab

                for i in range(lt):
                    def u_out(i=i):
                        cs = off + i * 128
                        ot = otp.tile([128, 1024], BF16, tag="ot")
                        for oi, (ooff, ow) in enumerate(blocks(D, 512)):
                            po = pmm.tile([128, 512], F32, tag="mm")
                            nc.tensor.matmul(po[:, 0:ow], vt[:, cs:cs + 128],
                                             w0s[:, ooff:ooff + ow],
                                             start=True, stop=True)
                            eng = oeng[0]
                            oeng[0] = (oeng[0] + 1) % 2
                            if eng == 0:
                                nc.vector.tensor_copy(ot[:, ooff:ooff + ow],
                                                      po[:, 0:ow])
                            else:
                                nc.scalar.copy(ot[:, ooff:ooff + ow], po[:, 0:ow])
                        nc.sync.dma_start(out_d[cs:cs + 128, :], ot[:])
                    yield u_out

            # ---------------- scores with woven epilogue ----------------
            def emit_scores(b, pending):
                off = offs[b]
                lt = lts[b]
                for i in range(lt):
                    N = (i + 1) * 128
                    if N <= 512:
                        # pack both heads in one PSUM tile at 512-aligned slot
                        # offsets (a matmul write must not cross a PSUM bank
                        # boundary): one exp + one 3-D reduce for the pair
                        w = N
                        sc = psc.tile([128, 1024], F32, tag="sc")
                        sc3 = sc.rearrange("p (s c) -> p s c", s=2)
                        for h in range(HL):
                            so = h * 512
                            nc.tensor.matmul(sc[:, so:so + w],
                                             qth[h][:, off + i * 128: off + N],
                                             kth[h][:, off: off + w],
                                             start=True, stop=False)
                            nc.tensor.matmul(sc[:, so + w - 128: so + w],
                                             negi[:], ub[:],
                                             start=False, stop=True,
                                             skip_group_check=True)
                        scr = srp.tile([128, 2, 512], BF16, tag="scr")
                        nc.scalar.activation(scr[:, 0:2, 0:w], sc3[:, 0:2, 0:w],
                                             AF.Exp)
                        nc.vector.tensor_reduce(dn[b][:, 2 * i: 2 * i + 2],
                                                scr[:, 0:2, 0:w], AX.X, ALU.add)
                    else:
                        for h in range(HL):
                            sc = psc.tile([128, 1024], F32, tag="sc")
                            bl = blocks(N, 512)
                            for bi, (boff, bw) in enumerate(bl):
                                last = (bi == len(bl) - 1)
                                nc.tensor.matmul(
                                    sc[:, boff:boff + bw],
                                    qth[h][:, off + i * 128: off + N],
                                    kth[h][:, off + boff: off + boff + bw],
                                    start=True, stop=not last)
                            nc.tensor.matmul(sc[:, N - 128:N], negi[:], ub[:],
                                             start=False, stop=True,
                                             skip_group_check=True)
                            scr = srp.tile([128, 1024], BF16, tag="scrw")
                            nc.scalar.activation(scr[:, 0:N], sc[:, 0:N], AF.Exp,
                                                 accum_out=dn[b][:, 2 * i + h:
                                                                 2 * i + h + 1])
                    emit_v_some(1)
                    for _ in range(2):
                        if pending:
                            pending.pop(0)()
                emit_prod(b)

            # ---------------- main schedule ----------------
            oeng = [0]
            pending = []
            for b in range(NB):
                emit_scores(b, pending)
                while pending:
                    pending.pop(0)()
                pending = list(epilogue_units(b, oeng))
            while pending:
                pending.pop(0)()

    nc.compile()
    return nc


def _get_nc(lts):
    key = tuple(lts)
    if key not in _CACHE:
        _CACHE[key] = _build(key)
    return _CACHE[key]


def _host_consts():
    aux = {}
    negi = np.zeros((128, 128), np.float32)
    np.fill_diagonal(negi, NEG)
    aux["negi"] = negi.astype(ml_dtypes.bfloat16)
    aux["ub"] = np.triu(np.ones((128, 128), np.float32), 1).astype(ml_dtypes.bfloat16)
    aux["idenf"] = np.eye(128, dtype=np.float32)
    aux["ones64"] = np.ones((64, 1), np.float32).astype(ml_dtypes.bfloat16)
    ind2 = np.zeros((33, 128), np.float32)
    ind2[0, 0:64] = 1.0
    ind2[32, 64:128] = 1.0
    aux["ind2"] = ind2.astype(ml_dtypes.bfloat16)
    return aux


def _run(inputs, trace=False):
    from concourse.bass_utils import run_bass_kernel_spmd

    batch = np.asarray(inputs["batch"], np.float32)
    lengths = np.asarray(inputs["lengths"]).astype(np.int64)
    assert batch.shape == (B, S, D), batch.shape
    lt_all = [max(1, int(np.ceil(int(l) / 128.0))) for l in lengths]
    order = sorted(range(B), key=lambda b: -lt_all[b])
    lts = tuple(lt_all[b] for b in order)
    offs = []
    o = 0
    for lt in lts:
        offs.append(o * 128)
        o += lt
    SC = o * 128

    nc = _get_nc(lts)

    XT = np.concatenate(
        [batch[order[k]][: lts[k] * 128, :].T for k in range(B)], axis=1)
    xt = np.ascontiguousarray(XT.reshape(D // 128, 128, SC)).astype(ml_dtypes.bfloat16)
    consts = _host_consts()
    wq = np.asarray(inputs["wq"], np.float32)
    wk = np.asarray(inputs["wk"], np.float32)
    wv = np.asarray(inputs["wv"], np.float32)
    w0 = np.asarray(inputs["w0"], np.float32)
    bqf = np.asarray(inputs["bq"], np.float32)
    bkf = np.asarray(inputs["bk"], np.float32)
    bvf = np.asarray(inputs["bv"], np.float32)

    in_maps = []
    for j in range(8):
        sl = slice(j * 128, (j + 1) * 128)
        im = dict(consts)
        im["xt"] = xt
        im["wq"] = np.ascontiguousarray(
            wq[:, sl].reshape(8, 128, 128)).astype(ml_dtypes.bfloat16)
        im["wk"] = np.ascontiguousarray(
            wk[:, sl].reshape(8, 128, 128)).astype(ml_dtypes.bfloat16)
        im["wv"] = np.ascontiguousarray(
            wv[:, sl].reshape(8, 128, 128)).astype(ml_dtypes.bfloat16)
        im["w0"] = np.ascontiguousarray(w0[sl, :]).astype(ml_dtypes.bfloat16)
        im["bq"] = np.ascontiguousarray(bqf[sl].reshape(128, 1))
        im["bk"] = np.ascontiguousarray(bkf[sl].reshape(128, 1))
        im["bv"] = np.ascontiguousarray(bvf[sl].reshape(128, 1))
        in_maps.append(im)

    res = run_bass_kernel_spmd(nc, in_maps, core_ids=list(range(8)), trace=trace)

    acc = np.zeros((SC, D), np.float32)
    for r in res.results:
        acc += np.asarray(r["out"]).astype(np.float32)
    b0 = np.asarray(inputs["b0"], np.float32)
    out = np.empty((B, S, D), np.float32)
    out[:] = b0[None, None, :]
    for k in range(B):
        b = order[k]
        L = int(lengths[b])
        out[b, :L, :] += acc[offs[k]: offs[k] + L, :]
    return out, res


def kernel(**inputs) -> np.ndarray:
    out, _ = _run(inputs, trace=False)
    return out


# revision 18
# speedup vs baseline: 1.1077x; 1.0365x over previous
"""Trainium2 Bass kernel for nn_MultiHeadAttention_85761906966848 (sparse_attention).

The reference module only uses the DIAGONAL of the softmax attention matrix:
    out[b,s,:] = (softmax(masked scores)[s,s] * v[b,s,:]) @ W0 + b0
so no attn @ V matmul is needed — only QK^T row-sums of exp (softmax
denominators), the diagonal q_s.k_s, and the four dense projections.

Sharding: TENSOR-PARALLEL over heads. Core j owns heads (2j, 2j+1):
  * Q/K/V projections restricted to that 128-wide feature slice for ALL
    batches, sequence axis trimmed to ceil(L_b/128)*128 valid rows.
  * Scores / softmax denominators / diagonal weights per local head.
  * O-projection uses the 128-row slice of W0 -> per-core PARTIAL outputs,
    summed on the host (linear combine), which also adds b0.
Uniform SPMD by construction; only the weight slices per in_map differ.

Matmuls in bf16; causal mask folded into the score matmul as an extra
(-1e30*I).T @ strict_upper_ones PSUM accumulation. Score tiles of width<=512
pack BOTH heads into one PSUM tile: one exp + one strided 3-D reduce per
pair, halving the scalar-engine op count on the critical softmax path.
Epilogue work of batch b-1 is woven between score tiles of batch b.
"""

import numpy as np
import ml_dtypes
import concourse.bass as bass
import concourse.bacc as bacc
import concourse.mybir as mybir
from concourse import tile

F32 = mybir.dt.float32
BF16 = mybir.dt.bfloat16
AF = mybir.ActivationFunctionType
AX = mybir.AxisListType
ALU = mybir.AluOpType

B, S, D, H = 8, 1024, 1024, 16
dk = D // H
HL = 2
NEG = -1.0e30

_CACHE = {}


def blocks(total, width):
    out = []
    off = 0
    while off < total:
        w = min(width, total - off)
        out.append((off, w))
        off += w
    return out


def _build(lts):
    """lts: tuple of per-batch 128-row tile counts, in processing order."""
    NB = len(lts)
    TC = sum(lts)
    SC = TC * 128
    offs = []
    o = 0
    for lt in lts:
        offs.append(o * 128)
        o += lt
    C = D // 128

    nc = bacc.Bacc("TRN2", target_bir_lowering=False, debug=False, num_devices=8)

    xt_d = nc.dram_tensor("xt", [C, 128, SC], BF16, kind="ExternalInput")
    wq_d = nc.dram_tensor("wq", [C, 128, 128], BF16, kind="ExternalInput")
    wk_d = nc.dram_tensor("wk", [C, 128, 128], BF16, kind="ExternalInput")
    wv_d = nc.dram_tensor("wv", [C, 128, 128], BF16, kind="ExternalInput")
    w0_d = nc.dram_tensor("w0", [128, D], BF16, kind="ExternalInput")
    bq_d = nc.dram_tensor("bq", [128, 1], F32, kind="ExternalInput")
    bk_d = nc.dram_tensor("bk", [128, 1], F32, kind="ExternalInput")
    bv_d = nc.dram_tensor("bv", [128, 1], F32, kind="ExternalInput")
    negi_d = nc.dram_tensor("negi", [128, 128], BF16, kind="ExternalInput")
    ub_d = nc.dram_tensor("ub", [128, 128], BF16, kind="ExternalInput")
    idenf_d = nc.dram_tensor("idenf", [128, 128], F32, kind="ExternalInput")
    ones64_d = nc.dram_tensor("ones64", [64, 1], BF16, kind="ExternalInput")
    ind2_d = nc.dram_tensor("ind2", [33, 128], BF16, kind="ExternalInput")
    out_d = nc.dram_tensor("out", [SC, D], BF16, kind="ExternalOutput")

    with tile.TileContext(nc) as tc:
        with (
            tc.tile_pool(name="cp", bufs=1) as cp,
            tc.tile_pool(name="xp", bufs=1) as xp,
            tc.tile_pool(name="qkp", bufs=1) as qkp,
            tc.tile_pool(name="srp", bufs=3) as srp,
            tc.tile_pool(name="prp", bufs=3) as prp,
            tc.tile_pool(name="dlp", bufs=2) as dlp,
            tc.tile_pool(name="otp", bufs=3) as otp,
            tc.tile_pool(name="psc", bufs=3, space=bass.MemorySpace.PSUM) as psc,
            tc.tile_pool(name="pmm", bufs=2, space=bass.MemorySpace.PSUM) as pmm,
        ):
            # ---------------- constants (sync queue) ----------------
            negi = cp.tile([128, 128], BF16, tag="negi")
            nc.sync.dma_start(negi[:], negi_d[:])
            ub = cp.tile([128, 128], BF16, tag="ub")
            nc.sync.dma_start(ub[:], ub_d[:])
            idenf = cp.tile([128, 128], F32, tag="idenf")
            nc.sync.dma_start(idenf[:], idenf_d[:])
            ones64 = cp.tile([64, 1], BF16, tag="ones64")
            nc.sync.dma_start(ones64[:], ones64_d[:])
            ind2 = cp.tile([33, 128], BF16, tag="ind2")
            nc.sync.dma_start(ind2[:], ind2_d[:])
            bq = cp.tile([128, 1], F32, tag="bq")
            nc.sync.dma_start(bq[:], bq_d[:])
            bk = cp.tile([128, 1], F32, tag="bk")
            nc.sync.dma_start(bk[:], bk_d[:])
            bv = cp.tile([128, 1], F32, tag="bv")
            nc.sync.dma_start(bv[:], bv_d[:])

            # weights on scalar queue first (small, needed first)
            wqs = [cp.tile([128, 128], BF16, name=f"wq{c}", tag=f"wq{c}") for c in range(C)]
            wks = [cp.tile([128, 128], BF16, name=f"wk{c}", tag=f"wk{c}") for c in range(C)]
            wvs = [cp.tile([128, 128], BF16, name=f"wv{c}", tag=f"wv{c}") for c in range(C)]
            for c in range(C):
                nc.scalar.dma_start(wks[c][:], wk_d[c, :, :])
            for c in range(C):
                nc.scalar.dma_start(wqs[c][:], wq_d[c, :, :])

            # X^T resident: narrow first block for a fast start, wide after
            xt = [xp.tile([128, SC], BF16, name=f"xt{c}", tag=f"xt{c}") for c in range(C)]
            di = 0
            for (boff, bw) in blocks(SC, 1024):
                for c in range(C):
                    eng = nc.sync if di % 2 == 0 else nc.scalar
                    eng.dma_start(xt[c][:, boff:boff + bw],
                                  xt_d[c, :, boff:boff + bw])
                    di += 1
            for c in range(C):
                nc.scalar.dma_start(wvs[c][:], wv_d[c, :, :])
            w0s = cp.tile([128, D], BF16, tag="w0s")
            nc.scalar.dma_start(w0s[:], w0_d[:])

            # persistent per-core tensors
            qth = [qkp.tile([64, SC], BF16, name=f"qth{h}", tag=f"qth{h}") for h in range(HL)]
            kth = [qkp.tile([64, SC], BF16, name=f"kth{h}", tag=f"kth{h}") for h in range(HL)]
            vt = qkp.tile([128, SC], BF16, tag="vt")
            a2 = cp.tile([33, SC], BF16, tag="a2")
            nc.vector.memset(a2[:], 0.0)
            dn = [cp.tile([128, 2 * lt], F32, name=f"dn{b}", tag=f"dn{b}")
                  for b, lt in enumerate(lts)]

            # -------- streaming K/Q/V projection, one 512 block at a time
            def drain_k(ps, boff, bw):
                nc.scalar.activation(kth[0][:, boff:boff + bw], ps[0:64, 0:bw],
                                     AF.Identity, bias=bk[0:64, :])
                nc.vector.tensor_scalar_add(kth[1][:, boff:boff + bw],
                                            ps[64:128, 0:bw], bk[64:128, :])

            def drain_q(ps, boff, bw):
                nc.vector.tensor_scalar_add(qth[0][:, boff:boff + bw],
                                            ps[0:64, 0:bw], bq[0:64, :])
                nc.scalar.activation(qth[1][:, boff:boff + bw], ps[64:128, 0:bw],
                                     AF.Identity, bias=bq[64:128, :])

            def drain_v(ps, boff, bw):
                nc.vector.tensor_scalar_add(vt[:, boff:boff + bw], ps[:, 0:bw],
                                            bv[:])

            pblocks = blocks(SC, 512)
            pstate = {"next": 0}

            def emit_proj_block():
                boff, bw = pblocks[pstate["next"]]
                pstate["next"] += 1
                for w_tiles, drain in ((wks, drain_k), (wqs, drain_q),
                                       (wvs, drain_v)):
                    ps = pmm.tile([128, 512], F32, tag="mm")
                    for kk in range(C):
                        nc.tensor.matmul(ps[:, 0:bw], w_tiles[kk][:],
                                         xt[kk][:, boff:boff + bw],
                                         start=(kk == 0), stop=(kk == C - 1))
                    drain(ps, boff, bw)

            def emit_proj_some(n):
                for _ in range(n):
                    if pstate["next"] < len(pblocks):
                        emit_proj_block()

            def emit_proj_upto(col_end):
                while (pstate["next"] < len(pblocks)
                       and pblocks[pstate["next"]][0] < col_end):
                    emit_proj_block()

            # ---------------- diag products (q*k hi/lo), per 512 block -----
            prod_tiles = {}

            def emit_prod(e):
                off = offs[e]
                scb = lts[e] * 128
                for h in range(HL):
                    for bi, (boff, bw) in enumerate(blocks(scb, 512)):
                        sli = slice(off + boff, off + boff + bw)
                        pr32 = prp.tile([64, 512], F32, tag="pr32")
                        nc.vector.tensor_mul(pr32[:, 0:bw], qth[h][:, sli],
                                             kth[h][:, sli])
                        prh = prp.tile([64, 512], BF16, name=f"prh{h}_{bi}",
                                       tag=f"prh{h}_{bi}", bufs=1)
                        nc.gpsimd.tensor_copy(prh[:, 0:bw], pr32[:, 0:bw])
                        prl = prp.tile([64, 512], BF16, name=f"prl{h}_{bi}",
                                       tag=f"prl{h}_{bi}", bufs=1)
                        nc.gpsimd.tensor_sub(prl[:, 0:bw], pr32[:, 0:bw],
                                             prh[:, 0:bw])
                        prod_tiles[(e, h, bi)] = (prh, prl, bw)

            # ------------- epilogue of batch e as a unit queue -------------
            def epilogue_units(e, oeng):
                off = offs[e]
                lt = lts[e]
                scb = lt * 128
                st = {}

                def u_recip():
                    rec = prp.tile([128, 16], F32, tag="rec")
                    nc.vector.reciprocal(rec[:, 0:2 * lt], dn[e][:])
                    tpr = pmm.tile([16, 128], F32, tag="mm")
                    nc.tensor.transpose(tpr[0:2 * lt, :], rec[:, 0:2 * lt], idenf[:])
                    recT = prp.tile([16, 128], BF16, tag="recT")
                    nc.vector.tensor_copy(recT[0:2 * lt, :], tpr[0:2 * lt, :])
                    st["arecl"] = [dlp.tile([1, 1024], BF16, name=f"arecl{h}",
                                            tag=f"arecl{h}") for h in range(HL)]
                    for h in range(HL):
                        nc.sync.dma_start(st["arecl"][h][:, 0:scb],
                                          recT[h:2 * lt:2, :])
                yield u_recip

                def u_diag():
                    st["dexpl"] = [dlp.tile([1, 1024], BF16, name=f"dexpl{h}",
                                            tag=f"dexpl{h}") for h in range(HL)]
                    for h in range(HL):
                        for bi, (boff, bw) in enumerate(blocks(scb, 512)):
                            prh, prl, _ = prod_tiles.pop((e, h, bi))
                            dg = pmm.tile([1, 512], F32, tag="mm")
                            nc.tensor.matmul(dg[:, 0:bw], ones64[:], prh[:, 0:bw],
                                             start=True, stop=False)
                            nc.tensor.matmul(dg[:, 0:bw], ones64[:], prl[:, 0:bw],
                                             start=False, stop=True)
                            nc.scalar.activation(st["dexpl"][h][:, boff:boff + bw],
                                                 dg[:, 0:bw], AF.Exp)
                yield u_diag

                def u_a2():
                    for h in range(HL):
                        nc.vector.tensor_mul(a2[32 * h:32 * h + 1, off:off + scb],
                                             st["dexpl"][h][:, 0:scb],
                                             st["arecl"][h][:, 0:scb])
                yield u_a2

                for (boff, bw) in blocks(scb, 512):
                    def u_ab(boff=boff, bw=bw):
                        ab = pmm.tile([128, 512], F32, tag="mm")
                        nc.tensor.matmul(ab[:, 0:bw], ind2[:],
                                         a2[:, off + boff: off + boff + bw],
                                         start=True, stop=True)
                        nc.vector.tensor_mul(vt[:, off + boff: off + boff + bw],
                                             vt[:, off + boff: off + boff + bw],
                                             ab[:, 0:bw])
                    yield u_ab

                for i in range(lt):
                    def u_out(i=i):
                        cs = off + i * 128
                        ot = otp.tile([128, 1024], BF16, tag="ot")
                        for oi, (ooff, ow) in enumerate(blocks(D, 512)):
                            po = pmm.tile([128, 512], F32, tag="mm")
                            nc.tensor.matmul(po[:, 0:ow], vt[:, cs:cs + 128],
                                             w0s[:, ooff:ooff + ow],
                                             start=True, stop=True)
                            eng = oeng[0]
                            oeng[0] = (oeng[0] + 1) % 2
                            if eng == 0:
                                nc.vector.tensor_copy(ot[:, ooff:ooff + ow],
                                                      po[:, 0:ow])
                            else:
                                nc.scalar.copy(ot[:, ooff:ooff + ow], po[:, 0:ow])
                        nc.sync.dma_start(out_d[cs:cs + 128, :], ot[:])
                    yield u_out

            # ---------------- scores with woven epilogue ----------------
            def emit_scores(b, pending):
                off = offs[b]
                lt = lts[b]
                emit_proj_upto(off + lt * 128)
                for i in range(lt):
                    emit_proj_some(1)
                    N = (i + 1) * 128
                    if N <= 512:
                        # pack both heads in one PSUM tile at 512-aligned slot
                        # offsets (a matmul write must not cross a PSUM bank
                        # boundary): one exp + one 3-D reduce for the pair
                        w = N
                        sc = psc.tile([128, 1024], F32, tag="sc")
                        sc3 = sc.rearrange("p (s c) -> p s c", s=2)
                        for h in range(HL):
                            so = h * 512
                            nc.tensor.matmul(sc[:, so:so + w],
                                             qth[h][:, off + i * 128: off + N],
                                             kth[h][:, off: off + w],
                                             start=True, stop=False)
                            nc.tensor.matmul(sc[:, so + w - 128: so + w],
                                             negi[:], ub[:],
                                             start=False, stop=True,
                                             skip_group_check=True)
                        scr = srp.tile([128, 2, 512], BF16, tag="scr")
                        nc.scalar.activation(scr[:, 0:2, 0:w], sc3[:, 0:2, 0:w],
                                             AF.Exp)
                        nc.vector.tensor_reduce(dn[b][:, 2 * i: 2 * i + 2],
                                                scr[:, 0:2, 0:w], AX.X, ALU.add)
                    else:
                        for h in range(HL):
                            sc = psc.tile([128, 1024], F32, tag="sc")
                            bl = blocks(N, 512)
                            for bi, (boff, bw) in enumerate(bl):
                                last = (bi == len(bl) - 1)
                                nc.tensor.matmul(
                                    sc[:, boff:boff + bw],
                                    qth[h][:, off + i * 128: off + N],
                                    kth[h][:, off + boff: off + boff + bw],
                                    start=True, stop=not last)
                            nc.tensor.matmul(sc[:, N - 128:N], negi[:], ub[:],
                                             start=False, stop=True,
                                             skip_group_check=True)
                            scr = srp.tile([128, 1024], BF16, tag="scrw")
                            nc.scalar.activation(scr[:, 0:N], sc[:, 0:N], AF.Exp,
                                                 accum_out=dn[b][:, 2 * i + h:
                                                                 2 * i + h + 1])
                    for _ in range(2):
                        if pending:
                            pending.pop(0)()
                emit_prod(b)

            # ---------------- main schedule ----------------
            oeng = [0]
            pending = []
            for b in range(NB):
                emit_scores(b, pending)
                while pending:
                    pending.pop(0)()
                pending = list(epilogue_units(b, oeng))
            while pending:
                pending.pop(0)()

    nc.compile()
    return nc


def _get_nc(lts):
    key = tuple(lts)
    if key not in _CACHE:
        _CACHE[key] = _build(key)
    return _CACHE[key]


def _host_consts():
    aux = {}
    negi = np.zeros((128, 128), np.float32)
    np.fill_diagonal(negi, NEG)
    aux["negi"] = negi.astype(ml_dtypes.bfloat16)
    aux["ub"] = np.triu(np.ones((128, 128), np.float32), 1).astype(ml_dtypes.bfloat16)
    aux["idenf"] = np.eye(128, dtype=np.float32)
    aux["ones64"] = np.ones((64, 1), np.float32).astype(ml_dtypes.bfloat16)
    ind2 = np.zeros((33, 128), np.float32)
    ind2[0, 0:64] = 1.0
    ind2[32, 64:128] = 1.0
    aux["ind2"] = ind2.astype(ml_dtypes.bfloat16)
    return aux


def _run(inputs, trace=False):
    from concourse.bass_utils import run_bass_kernel_spmd

    batch = np.asarray(inputs["batch"], np.float32)
    lengths = np.asarray(inputs["lengths"]).astype(np.int64)
    assert batch.shape == (B, S, D), batch.shape
    lt_all = [max(1, int(np.ceil(int(l) / 128.0))) for l in lengths]
    order = sorted(range(B), key=lambda b: -lt_all[b])
    lts = tuple(lt_all[b] for b in order)
    offs = []
    o = 0
    for lt in lts:
        offs.append(o * 128)
        o += lt
    SC = o * 128

    nc = _get_nc(lts)

    XT = np.concatenate(
        [batch[order[k]][: lts[k] * 128, :].T for k in range(B)], axis=1)
    xt = np.ascontiguousarray(XT.reshape(D // 128, 128, SC)).astype(ml_dtypes.bfloat16)
    consts = _host_consts()
    wq = np.asarray(inputs["wq"], np.float32)
    wk = np.asarray(inputs["wk"], np.float32)
    wv = np.asarray(inputs["wv"], np.float32)
    w0 = np.asarray(inputs["w0"], np.float32)
    bqf = np.asarray(inputs["bq"], np.float32)
    bkf = np.asarray(inputs["bk"], np.float32)
    bvf = np.asarray(inputs["bv"], np.float32)

    in_maps = []
    for j in range(8):
        sl = slice(j * 128, (j + 1) * 128)
        im = dict(consts)
        im["xt"] = xt
        im["wq"] = np.ascontiguousarray(
            wq[:, sl].reshape(8, 128, 128)).astype(ml_dtypes.bfloat16)
        im["wk"] = np.ascontiguousarray(
            wk[:, sl].reshape(8, 128, 128)).astype(ml_dtypes.bfloat16)
        im["wv"] = np.ascontiguousarray(
            wv[:, sl].reshape(8, 128, 128)).astype(ml_dtypes.bfloat16)
        im["w0"] = np.ascontiguousarray(w0[sl, :]).astype(ml_dtypes.bfloat16)
        im["bq"] = np.ascontiguousarray(bqf[sl].reshape(128, 1))
        im["bk"] = np.ascontiguousarray(bkf[sl].reshape(128, 1))
        im["bv"] = np.ascontiguousarray(bvf[sl].reshape(128, 1))
        in_maps.append(im)

    res = run_bass_kernel_spmd(nc, in_maps, core_ids=list(range(8)), trace=trace)

    acc = np.zeros((SC, D), np.float32)
    for r in res.results:
        acc += np.asarray(r["out"]).astype(np.float32)
    b0 = np.asarray(inputs["b0"], np.float32)
    out = np.empty((B, S, D), np.float32)
    out[:] = b0[None, None, :]
    for k in range(B):
        b = order[k]
        L = int(lengths[b])
        out[b, :L, :] += acc[offs[k]: offs[k] + L, :]
    return out, res


def kernel(**inputs) -> np.ndarray:
    out, _ = _run(inputs, trace=False)
    return out


# revision 19
# speedup vs baseline: 1.1147x; 1.0063x over previous
"""Trainium2 Bass kernel for nn_MultiHeadAttention_85761906966848 (sparse_attention).

The reference module only uses the DIAGONAL of the softmax attention matrix:
    out[b,s,:] = (softmax(masked scores)[s,s] * v[b,s,:]) @ W0 + b0
so no attn @ V matmul is needed — only QK^T row-sums of exp (softmax
denominators), the diagonal q_s.k_s, and the four dense projections.

Sharding: TENSOR-PARALLEL over heads. Core j owns heads (2j, 2j+1):
  * Q/K/V projections restricted to that 128-wide feature slice for ALL
    batches, sequence axis trimmed to ceil(L_b/128)*128 valid rows.
  * Scores / softmax denominators / diagonal weights per local head.
  * O-projection uses the 128-row slice of W0 -> per-core PARTIAL outputs,
    summed on the host (linear combine), which also adds b0.
Uniform SPMD by construction; only the weight slices per in_map differ.

Matmuls in bf16; causal mask folded into the score matmul as an extra
(-1e30*I).T @ strict_upper_ones PSUM accumulation. Score tiles of width<=512
pack BOTH heads into one PSUM tile: one exp + one strided 3-D reduce per
pair, halving the scalar-engine op count on the critical softmax path.
Epilogue work of batch b-1 is woven between score tiles of batch b.
"""

import numpy as np
import ml_dtypes
import concourse.bass as bass
import concourse.bacc as bacc
import concourse.mybir as mybir
from concourse import tile

F32 = mybir.dt.float32
BF16 = mybir.dt.bfloat16
AF = mybir.ActivationFunctionType
AX = mybir.AxisListType
ALU = mybir.AluOpType

B, S, D, H = 8, 1024, 1024, 16
dk = D // H
HL = 2
NEG = -1.0e30

_CACHE = {}


def blocks(total, width):
    out = []
    off = 0
    while off < total:
        w = min(width, total - off)
        out.append((off, w))
        off += w
    return out


def _build(lts):
    """lts: tuple of per-batch 128-row tile counts, in processing order."""
    NB = len(lts)
    TC = sum(lts)
    SC = TC * 128
    offs = []
    o = 0
    for lt in lts:
        offs.append(o * 128)
        o += lt
    C = D // 128

    nc = bacc.Bacc("TRN2", target_bir_lowering=False, debug=False, num_devices=8)

    xt_d = nc.dram_tensor("xt", [C, 128, SC], BF16, kind="ExternalInput")
    wq_d = nc.dram_tensor("wq", [C, 128, 128], BF16, kind="ExternalInput")
    wk_d = nc.dram_tensor("wk", [C, 128, 128], BF16, kind="ExternalInput")
    wv_d = nc.dram_tensor("wv", [C, 128, 128], BF16, kind="ExternalInput")
    w0_d = nc.dram_tensor("w0", [128, D], BF16, kind="ExternalInput")
    bq_d = nc.dram_tensor("bq", [128, 1], F32, kind="ExternalInput")
    bk_d = nc.dram_tensor("bk", [128, 1], F32, kind="ExternalInput")
    bv_d = nc.dram_tensor("bv", [128, 1], F32, kind="ExternalInput")
    negi_d = nc.dram_tensor("negi", [128, 128], BF16, kind="ExternalInput")
    ub_d = nc.dram_tensor("ub", [128, 128], BF16, kind="ExternalInput")
    idenf_d = nc.dram_tensor("idenf", [128, 128], F32, kind="ExternalInput")
    ones64_d = nc.dram_tensor("ones64", [64, 1], BF16, kind="ExternalInput")
    ind2_d = nc.dram_tensor("ind2", [33, 128], BF16, kind="ExternalInput")
    out_d = nc.dram_tensor("out", [SC, D], BF16, kind="ExternalOutput")

    with tile.TileContext(nc) as tc:
        with (
            tc.tile_pool(name="cp", bufs=1) as cp,
            tc.tile_pool(name="xp", bufs=1) as xp,
            tc.tile_pool(name="qkp", bufs=1) as qkp,
            tc.tile_pool(name="srp", bufs=3) as srp,
            tc.tile_pool(name="prp", bufs=3) as prp,
            tc.tile_pool(name="dlp", bufs=2) as dlp,
            tc.tile_pool(name="otp", bufs=3) as otp,
            tc.tile_pool(name="psc", bufs=3, space=bass.MemorySpace.PSUM) as psc,
            tc.tile_pool(name="pmm", bufs=2, space=bass.MemorySpace.PSUM) as pmm,
        ):
            # ---------------- constants (sync queue) ----------------
            negi = cp.tile([128, 128], BF16, tag="negi")
            nc.sync.dma_start(negi[:], negi_d[:])
            ub = cp.tile([128, 128], BF16, tag="ub")
            nc.sync.dma_start(ub[:], ub_d[:])
            idenf = cp.tile([128, 128], F32, tag="idenf")
            nc.sync.dma_start(idenf[:], idenf_d[:])
            ones64 = cp.tile([64, 1], BF16, tag="ones64")
            nc.sync.dma_start(ones64[:], ones64_d[:])
            ind2 = cp.tile([33, 128], BF16, tag="ind2")
            nc.sync.dma_start(ind2[:], ind2_d[:])
            bq = cp.tile([128, 1], F32, tag="bq")
            nc.sync.dma_start(bq[:], bq_d[:])
            bk = cp.tile([128, 1], F32, tag="bk")
            nc.sync.dma_start(bk[:], bk_d[:])
            bv = cp.tile([128, 1], F32, tag="bv")
            nc.sync.dma_start(bv[:], bv_d[:])

            # weights on scalar queue first (small, needed first)
            wqs = [cp.tile([128, 128], BF16, name=f"wq{c}", tag=f"wq{c}") for c in range(C)]
            wks = [cp.tile([128, 128], BF16, name=f"wk{c}", tag=f"wk{c}") for c in range(C)]
            wvs = [cp.tile([128, 128], BF16, name=f"wv{c}", tag=f"wv{c}") for c in range(C)]
            for c in range(C):
                nc.scalar.dma_start(wks[c][:], wk_d[c, :, :])
            for c in range(C):
                nc.scalar.dma_start(wqs[c][:], wq_d[c, :, :])

            # X^T resident: narrow first block for a fast start, wide after
            xt = [xp.tile([128, SC], BF16, name=f"xt{c}", tag=f"xt{c}") for c in range(C)]
            di = 0
            xengs = [nc.sync, nc.scalar, nc.gpsimd]
            for (boff, bw) in blocks(SC, 1024):
                for c in range(C):
                    xengs[di % 3].dma_start(xt[c][:, boff:boff + bw],
                                            xt_d[c, :, boff:boff + bw])
                    di += 1
            for c in range(C):
                nc.scalar.dma_start(wvs[c][:], wv_d[c, :, :])
            w0s = cp.tile([128, D], BF16, tag="w0s")
            nc.scalar.dma_start(w0s[:], w0_d[:])

            # persistent per-core tensors
            qth = [qkp.tile([64, SC], BF16, name=f"qth{h}", tag=f"qth{h}") for h in range(HL)]
            kth = [qkp.tile([64, SC], BF16, name=f"kth{h}", tag=f"kth{h}") for h in range(HL)]
            vt = qkp.tile([128, SC], BF16, tag="vt")
            a2 = cp.tile([33, SC], BF16, tag="a2")
            nc.vector.memset(a2[:], 0.0)
            dn = [cp.tile([128, 2 * lt], F32, name=f"dn{b}", tag=f"dn{b}")
                  for b, lt in enumerate(lts)]

            # -------- streaming K/Q/V projection, one 512 block at a time
            def drain_k(ps, boff, bw):
                nc.scalar.activation(kth[0][:, boff:boff + bw], ps[0:64, 0:bw],
                                     AF.Identity, bias=bk[0:64, :])
                nc.vector.tensor_scalar_add(kth[1][:, boff:boff + bw],
                                            ps[64:128, 0:bw], bk[64:128, :])

            def drain_q(ps, boff, bw):
                nc.vector.tensor_scalar_add(qth[0][:, boff:boff + bw],
                                            ps[0:64, 0:bw], bq[0:64, :])
                nc.scalar.activation(qth[1][:, boff:boff + bw], ps[64:128, 0:bw],
                                     AF.Identity, bias=bq[64:128, :])

            def drain_v(ps, boff, bw):
                nc.vector.tensor_scalar_add(vt[:, boff:boff + bw], ps[:, 0:bw],
                                            bv[:])

            pblocks = blocks(SC, 512)
            pstate = {"next": 0}

            def emit_proj_block():
                boff, bw = pblocks[pstate["next"]]
                pstate["next"] += 1
                for w_tiles, drain in ((wks, drain_k), (wqs, drain_q)):
                    ps = pmm.tile([128, 512], F32, tag="mm")
                    for kk in range(C):
                        nc.tensor.matmul(ps[:, 0:bw], w_tiles[kk][:],
                                         xt[kk][:, boff:boff + bw],
                                         start=(kk == 0), stop=(kk == C - 1))
                    drain(ps, boff, bw)

            def emit_proj_some(n):
                for _ in range(n):
                    if pstate["next"] < len(pblocks):
                        emit_proj_block()

            def emit_proj_upto(col_end):
                while (pstate["next"] < len(pblocks)
                       and pblocks[pstate["next"]][0] < col_end):
                    emit_proj_block()

            vstate = {"next": 0}

            def emit_v_block():
                boff, bw = pblocks[vstate["next"]]
                vstate["next"] += 1
                ps = pmm.tile([128, 512], F32, tag="mm")
                for kk in range(C):
                    nc.tensor.matmul(ps[:, 0:bw], wvs[kk][:],
                                     xt[kk][:, boff:boff + bw],
                                     start=(kk == 0), stop=(kk == C - 1))
                drain_v(ps, boff, bw)

            def emit_v_some(n):
                for _ in range(n):
                    if vstate["next"] < len(pblocks):
                        emit_v_block()

            def emit_v_upto(col_end):
                while (vstate["next"] < len(pblocks)
                       and pblocks[vstate["next"]][0] < col_end):
                    emit_v_block()

            # ---------------- diag products (q*k hi/lo), per 512 block -----
            prod_tiles = {}

            def emit_prod(e):
                off = offs[e]
                scb = lts[e] * 128
                for h in range(HL):
                    for bi, (boff, bw) in enumerate(blocks(scb, 512)):
                        sli = slice(off + boff, off + boff + bw)
                        pr32 = prp.tile([64, 512], F32, tag="pr32")
                        nc.vector.tensor_mul(pr32[:, 0:bw], qth[h][:, sli],
                                             kth[h][:, sli])
                        prh = prp.tile([64, 512], BF16, name=f"prh{h}_{bi}",
                                       tag=f"prh{h}_{bi}", bufs=1)
                        nc.gpsimd.tensor_copy(prh[:, 0:bw], pr32[:, 0:bw])
                        prl = prp.tile([64, 512], BF16, name=f"prl{h}_{bi}",
                                       tag=f"prl{h}_{bi}", bufs=1)
                        nc.gpsimd.tensor_sub(prl[:, 0:bw], pr32[:, 0:bw],
                                             prh[:, 0:bw])
                        prod_tiles[(e, h, bi)] = (prh, prl, bw)

            # ------------- epilogue of batch e as a unit queue -------------
            def epilogue_units(e, oeng):
                off = offs[e]
                lt = lts[e]
                scb = lt * 128
                st = {}

                def u_recip():
                    emit_v_upto(off + scb)
                    rec = prp.tile([128, 16], F32, tag="rec")
                    nc.vector.reciprocal(rec[:, 0:2 * lt], dn[e][:])
                    tpr = pmm.tile([16, 128], F32, tag="mm")
                    nc.tensor.transpose(tpr[0:2 * lt, :], rec[:, 0:2 * lt], idenf[:])
                    recT = prp.tile([16, 128], BF16, tag="recT")
                    nc.vector.tensor_copy(recT[0:2 * lt, :], tpr[0:2 * lt, :])
                    st["arecl"] = [dlp.tile([1, 1024], BF16, name=f"arecl{h}",
                                            tag=f"arecl{h}") for h in range(HL)]
                    for h in range(HL):
                        nc.sync.dma_start(st["arecl"][h][:, 0:scb],
                                          recT[h:2 * lt:2, :])
                yield u_recip

                def u_diag():
                    st["dexpl"] = [dlp.tile([1, 1024], BF16, name=f"dexpl{h}",
                                            tag=f"dexpl{h}") for h in range(HL)]
                    for h in range(HL):
                        for bi, (boff, bw) in enumerate(blocks(scb, 512)):
                            prh, prl, _ = prod_tiles.pop((e, h, bi))
                            dg = pmm.tile([1, 512], F32, tag="mm")
                            nc.tensor.matmul(dg[:, 0:bw], ones64[:], prh[:, 0:bw],
                                             start=True, stop=False)
                            nc.tensor.matmul(dg[:, 0:bw], ones64[:], prl[:, 0:bw],
                                             start=False, stop=True)
                            nc.scalar.activation(st["dexpl"][h][:, boff:boff + bw],
                                                 dg[:, 0:bw], AF.Exp)
                yield u_diag

                def u_a2():
                    for h in range(HL):
                        nc.vector.tensor_mul(a2[32 * h:32 * h + 1, off:off + scb],
                                             st["dexpl"][h][:, 0:scb],
                                             st["arecl"][h][:, 0:scb])
                yield u_a2

                for (boff, bw) in blocks(scb, 512):
                    def u_ab(boff=boff, bw=bw):
                        ab = pmm.tile([128, 512], F32, tag="mm")
                        nc.tensor.matmul(ab[:, 0:bw], ind2[:],
                                         a2[:, off + boff: off + boff + bw],
                                         start=True, stop=True)
                        nc.vector.tensor_mul(vt[:, off + boff: off + boff + bw],
                                             vt[:, off + boff: off + boff + bw],
                                             ab[:, 0:bw])
                    yield u_ab

                for i in range(lt):
                    def u_out(i=i):
                        cs = off + i * 128
                        ot = otp.tile([128, 1024], BF16, tag="ot")
                        for oi, (ooff, ow) in enumerate(blocks(D, 512)):
                            po = pmm.tile([128, 512], F32, tag="mm")
                            nc.tensor.matmul(po[:, 0:ow], vt[:, cs:cs + 128],
                                             w0s[:, ooff:ooff + ow],
                                             start=True, stop=True)
                            eng = oeng[0]
                            oeng[0] = (oeng[0] + 1) % 2
                            if eng == 0:
                                nc.vector.tensor_copy(ot[:, ooff:ooff + ow],
                                                      po[:, 0:ow])
                            else:
                                nc.scalar.copy(ot[:, ooff:ooff + ow], po[:, 0:ow])
                        nc.sync.dma_start(out_d[cs:cs + 128, :], ot[:])
                    yield u_out

            # ---------------- scores with woven epilogue ----------------
            def emit_scores(b, pending):
                off = offs[b]
                lt = lts[b]
                emit_proj_upto(off + lt * 128)
                for i in range(lt):
                    emit_proj_some(1)
                    emit_v_some(1)
                    N = (i + 1) * 128
                    if N <= 512:
                        # pack both heads in one PSUM tile at 512-aligned slot
                        # offsets (a matmul write must not cross a PSUM bank
                        # boundary): one exp + one 3-D reduce for the pair
                        w = N
                        sc = psc.tile([128, 1024], F32, tag="sc")
                        sc3 = sc.rearrange("p (s c) -> p s c", s=2)
                        for h in range(HL):
                            so = h * 512
                            nc.tensor.matmul(sc[:, so:so + w],
                                             qth[h][:, off + i * 128: off + N],
                                             kth[h][:, off: off + w],
                                             start=True, stop=False)
                            nc.tensor.matmul(sc[:, so + w - 128: so + w],
                                             negi[:], ub[:],
                                             start=False, stop=True,
                                             skip_group_check=True)
                        scr = srp.tile([128, 2, 512], BF16, tag="scr")
                        nc.scalar.activation(scr[:, 0:2, 0:w], sc3[:, 0:2, 0:w],
                                             AF.Exp)
                        nc.vector.tensor_reduce(dn[b][:, 2 * i: 2 * i + 2],
                                                scr[:, 0:2, 0:w], AX.X, ALU.add)
                    else:
                        for h in range(HL):
                            sc = psc.tile([128, 1024], F32, tag="sc")
                            bl = blocks(N, 512)
                            for bi, (boff, bw) in enumerate(bl):
                                last = (bi == len(bl) - 1)
                                nc.tensor.matmul(
                                    sc[:, boff:boff + bw],
                                    qth[h][:, off + i * 128: off + N],
                                    kth[h][:, off + boff: off + boff + bw],
                                    start=True, stop=not last)
                            nc.tensor.matmul(sc[:, N - 128:N], negi[:], ub[:],
                                             start=False, stop=True,
                                             skip_group_check=True)
                            scr = srp.tile([128, 1024], BF16, tag="scrw")
                            nc.scalar.activation(scr[:, 0:N], sc[:, 0:N], AF.Exp,
                                                 accum_out=dn[b][:, 2 * i + h:
                                                                 2 * i + h + 1])
                    for _ in range(3):
                        if pending:
                            pending.pop(0)()
                emit_prod(b)

            # ---------------- main schedule ----------------
            oeng = [0]
            pending = []
            for b in range(NB):
                emit_scores(b, pending)
                while pending:
                    pending.pop(0)()
                pending = list(epilogue_units(b, oeng))
            while pending:
                pending.pop(0)()

    nc.compile()
    return nc


def _get_nc(lts):
    key = tuple(lts)
    if key not in _CACHE:
        _CACHE[key] = _build(key)
    return _CACHE[key]


def _host_consts():
    aux = {}
    negi = np.zeros((128, 128), np.float32)
    np.fill_diagonal(negi, NEG)
    aux["negi"] = negi.astype(ml_dtypes.bfloat16)
    aux["ub"] = np.triu(np.ones((128, 128), np.float32), 1).astype(ml_dtypes.bfloat16)
    aux["idenf"] = np.eye(128, dtype=np.float32)
    aux["ones64"] = np.ones((64, 1), np.float32).astype(ml_dtypes.bfloat16)
    ind2 = np.zeros((33, 128), np.float32)
    ind2[0, 0:64] = 1.0
    ind2[32, 64:128] = 1.0
    aux["ind2"] = ind2.astype(ml_dtypes.bfloat16)
    return aux


def _run(inputs, trace=False):
    from concourse.bass_utils import run_bass_kernel_spmd

    batch = np.asarray(inputs["batch"], np.float32)
    lengths = np.asarray(inputs["lengths"]).astype(np.int64)
    assert batch.shape == (B, S, D), batch.shape
    lt_all = [max(1, int(np.ceil(int(l) / 128.0))) for l in lengths]
    order = sorted(range(B), key=lambda b: -lt_all[b])
    lts = tuple(lt_all[b] for b in order)
    offs = []
    o = 0
    for lt in lts:
        offs.append(o * 128)
        o += lt
    SC = o * 128

    nc = _get_nc(lts)

    XT = np.concatenate(
        [batch[order[k]][: lts[k] * 128, :].T for k in range(B)], axis=1)
    xt = np.ascontiguousarray(XT.reshape(D // 128, 128, SC)).astype(ml_dtypes.bfloat16)
    consts = _host_consts()
    wq = np.asarray(inputs["wq"], np.float32)
    wk = np.asarray(inputs["wk"], np.float32)
    wv = np.asarray(inputs["wv"], np.float32)
    w0 = np.asarray(inputs["w0"], np.float32)
    bqf = np.asarray(inputs["bq"], np.float32)
    bkf = np.asarray(inputs["bk"], np.float32)
    bvf = np.asarray(inputs["bv"], np.float32)

    in_maps = []
    for j in range(8):
        sl = slice(j * 128, (j + 1) * 128)
        im = dict(consts)
        im["xt"] = xt
        im["wq"] = np.ascontiguousarray(
            wq[:, sl].reshape(8, 128, 128)).astype(ml_dtypes.bfloat16)
        im["wk"] = np.ascontiguousarray(
            wk[:, sl].reshape(8, 128, 128)).astype(ml_dtypes.bfloat16)
        im["wv"] = np.ascontiguousarray(
            wv[:, sl].reshape(8, 128, 128)).astype(ml_dtypes.bfloat16)
        im["w0"] = np.ascontiguousarray(w0[sl, :]).astype(ml_dtypes.bfloat16)
        im["bq"] = np.ascontiguousarray(bqf[sl].reshape(128, 1))
        im["bk"] = np.ascontiguousarray(bkf[sl].reshape(128, 1))
        im["bv"] = np.ascontiguousarray(bvf[sl].reshape(128, 1))
        in_maps.append(im)

    res = run_bass_kernel_spmd(nc, in_maps, core_ids=list(range(8)), trace=trace)

    acc = np.zeros((SC, D), np.float32)
    for r in res.results:
        acc += np.asarray(r["out"]).astype(np.float32)
    b0 = np.asarray(inputs["b0"], np.float32)
    out = np.empty((B, S, D), np.float32)
    out[:] = b0[None, None, :]
    for k in range(B):
        b = order[k]
        L = int(lengths[b])
        out[b, :L, :] += acc[offs[k]: offs[k] + L, :]
    return out, res


def kernel(**inputs) -> np.ndarray:
    out, _ = _run(inputs, trace=False)
    return out
